# revision 24
# baseline (speedup 1.0000x reference)
"""Trainium2 Bass kernel for nn_GAT_42786464203341.

8-way tensor parallel (Megatron-style) over one trn2 chip:
  - GAT edges are block-diagonal fully-connected per sample, so message
    passing is dense per-sample attention with scores leaky(el[i] + er[j]),
    softmaxed over source i (exp needs no max-subtraction, scores O(1)).
  - Activations feature-major (x^T: [D, nodes]); both samples fused on the
    free axis ([128, 532] SBUF tiles, per-266 psum chunks).
  - LayerNorm gains are folded into the weights host-side; the mean term and
    LN bias enter each consumer GEMM as K=1 rank-1 correction matmuls, so LN
    application is a single fused scale-and-cast (x * rsigma -> fp16) per
    k-tile instead of mul+sub+affine.
  - All weights fp16 (full PE rate, FWL weight loads, half the HBM bytes).
    Attention inner math stays f32r.  Matmul accumulation is fp32 PSUM.
  - Attention head-parallel (2 heads/core); W_proj row-sharded -> partial
    [D, nodes] -> AllReduce (fp16).  FFN column/row sharded -> AllReduce.
    Head vocab-sharded; host concatenates the 8 logits slices.
"""

import time
from contextlib import ExitStack

import ml_dtypes
import numpy as np

import concourse.bass as bass
import concourse.tile as tile
from concourse import bacc, mybir
from concourse.masks import make_identity

F32 = mybir.dt.float32
F32R = mybir.dt.float32r
F16 = mybir.dt.float16

B, T, NOBJ = 2, 265, 9
D, H, DH = 1536, 16, 96
V, PV, L, FF = 8192, 512, 3, 6144
N = B * T          # 530
NC = 8             # cores
HPC = H // NC      # heads per core
FFL = FF // NC     # 768
VL = V // NC       # 1024
NCH = T + 1        # 266 (col 265 of each chunk is zero padding)
NP = B * NCH       # 532
KD = D // 128      # 12
KF = FFL // 128    # 6
MTS = [[(0, 128), (128, 128), (256, 10)],
       [(266, 128), (394, 128), (522, 10)]]    # node tiles (fused offsets)
MT_REAL = [128, 128, 9]                        # non-pad rows per node tile
EPS = 1e-5

_CACHE = {}


# --------------------------------------------------------------------------
# host-side input prep
# --------------------------------------------------------------------------

def _block_diag_edges_np():
    base = np.arange(T)
    src = np.concatenate([g * T + np.repeat(base, T) for g in range(B)])
    dst = np.concatenate([g * T + np.tile(base, T) for g in range(B)])
    return src.astype(np.int64), dst.astype(np.int64)


def _host_inputs(inp, ffn_bf16=True):
    f32, f16 = np.float32, np.float16
    objs_e = np.asarray(inp["obj_emb_w"])[np.asarray(inp["objs"])]
    pe = np.asarray(inp["poss_emb_w"])[np.asarray(inp["poss"])]
    nfeat = np.concatenate([objs_e, pe[:, :NOBJ], pe[:, NOBJ:]], axis=-1)
    z = np.asarray(inp["tok_emb"])[np.asarray(inp["z_indices"])]
    x0 = np.concatenate([nfeat, z], axis=1) + np.asarray(inp["pos_emb"])[:, :T]
    x0 = x0.reshape(N, D).astype(f32)

    x0t = np.zeros((D, NP), f16)
    for b in range(B):
        x0t[:, b * NCH:b * NCH + T] = x0[b * T:(b + 1) * T].T.astype(f16)

    W_attn = np.asarray(inp["W_attn"], f32)
    a_l = np.asarray(inp["a_l"], f32)
    a_r = np.asarray(inp["a_r"], f32)
    W_proj = np.asarray(inp["W_proj"], f32)
    W_fc = np.asarray(inp["W_fc"], f32)
    W_out = np.asarray(inp["W_out"], f32)
    head_w = np.asarray(inp["head_w"], f32)
    g1 = np.asarray(inp["ln1_g"], f32)      # [L, D]
    b1 = np.asarray(inp["ln1_b"], f32)
    g2 = np.asarray(inp["ln2_g"], f32)
    b2 = np.asarray(inp["ln2_b"], f32)
    gf = np.asarray(inp["lnf_g"], f32)      # [D]
    bf = np.asarray(inp["lnf_b"], f32)

    def cols(vec, k_tiles):  # [3, D'] -> [3, 128, k_tiles]
        v = np.asarray(vec, f32)
        return np.transpose(v.reshape(3, k_tiles, 128), (0, 2, 1)).copy()

    bout8 = cols(np.asarray(inp["b_out"], f32) / NC, KD)
    bproj8 = cols(np.asarray(inp["b_proj"], f32) / NC, KD)

    maps = []
    for c in range(NC):
        h0 = c * HPC
        wattn = np.zeros((L, D, 256), f32)
        for j in range(HPC):
            hg = h0 + j
            blk = W_attn[:, :, hg * DH:(hg + 1) * DH]         # [3, D, DH]
            wattn[:, :, j * DH:(j + 1) * DH] = blk
            # el/er are linear in h: fold (W_attn-block @ a) into one column
            wattn[:, :, 192 + j] = np.matmul(blk, a_l[:, hg, :, None])[..., 0]
            wattn[:, :, 194 + j] = np.matmul(blk, a_r[:, hg, :, None])[..., 0]
        # LN1 fold: gain into weights; Gsum/Beta correction rows
        wattn_g = wattn * g1[:, :, None]                      # [L, D, 256]
        wa_gsum = np.einsum("ld,ldc->lc", g1, wattn)          # [L, 256]
        wa_beta = np.einsum("ld,ldc->lc", b1, wattn)          # [L, 256]

        wproj = np.stack(
            [W_proj[:, (h0 + j) * DH:(h0 + j + 1) * DH, :] for j in range(HPC)],
            axis=1,
        )
        wfc_sl = W_fc[:, :, c * FFL:(c + 1) * FFL]            # [L, D, FFL]
        wfc_g = wfc_sl * g2[:, :, None]
        wfc_gsum = np.einsum("ld,ldm->lm", g2, wfc_sl)        # [L, FFL]
        # LN2 beta folds into the fc bias column
        bfc_eff = np.asarray(inp["b_fc"], f32)[:, c * FFL:(c + 1) * FFL] \
            + np.einsum("ld,ldm->lm", b2, wfc_sl)             # [L, FFL]
        bfc_cols = np.transpose(bfc_eff.reshape(L, KF, 128), (0, 2, 1)).copy()

        whead_sl = head_w[:, c * VL:(c + 1) * VL]             # [D, VL]
        whead_g = whead_sl * gf[:, None]
        whead_gsum = (gf @ whead_sl)[None, :]                 # [1, VL]
        headb = (bf @ whead_sl).reshape(VL // 128, 128).T.copy()  # [128, 8]

        maps.append({
            "x0t": x0t,
            "wattn": wattn_g.astype(f16),
            "wa_gsum": wa_gsum[:, None, :].astype(f16),       # [L, 1, 256]
            "wa_beta": wa_beta[:, None, :].astype(f16),
            "wproj": np.ascontiguousarray(wproj).astype(f16),
            "wfc": np.ascontiguousarray(wfc_g).astype(f16),
            "wfc_gsum": wfc_gsum[:, None, :].astype(f16),     # [L, 1, FFL]
            "wout": np.ascontiguousarray(W_out[:, c * FFL:(c + 1) * FFL, :]).astype(f16),
            "whead": np.ascontiguousarray(whead_g).astype(f16),
            "whead_gsum": whead_gsum.astype(f16),             # [1, VL]
            "headb": headb,                                   # [128, 8] f32
            "ones_col": np.ones((128, 1), f32),
            "ones_row": np.ones((1, 128), f32),
            "bfc": np.ascontiguousarray(bfc_cols[..., None]), # [L, 128, KF, 1]->?
            "bout8": bout8, "bproj8": bproj8,
        })
    # fix bfc shape: want [L, KF, 128, 1]
    for m in maps:
        b = m["bfc"][..., 0]                                  # [L, 128, KF]
        m["bfc"] = np.ascontiguousarray(np.transpose(b, (0, 2, 1))[..., None])
    return maps


# --------------------------------------------------------------------------
# device program
# --------------------------------------------------------------------------

def _build_nc(reps=1, use_cc=True, ffn_bf16=True):
    nc = bacc.Bacc("TRN2", target_bir_lowering=False, debug=False, num_devices=NC)

    d_x0t = nc.declare_dram_parameter("x0t", [D, NP], F16, isOutput=False)
    d_wattn = nc.declare_dram_parameter("wattn", [L, D, 256], F16, isOutput=False)
    d_wagsum = nc.declare_dram_parameter("wa_gsum", [L, 1, 256], F16, isOutput=False)
    d_wabeta = nc.declare_dram_parameter("wa_beta", [L, 1, 256], F16, isOutput=False)
    d_wproj = nc.declare_dram_parameter("wproj", [L, HPC, DH, D], F16, isOutput=False)
    d_wfc = nc.declare_dram_parameter("wfc", [L, D, FFL], F16, isOutput=False)
    d_wfcgsum = nc.declare_dram_parameter("wfc_gsum", [L, 1, FFL], F16, isOutput=False)
    d_wout = nc.declare_dram_parameter("wout", [L, FFL, D], F16, isOutput=False)
    d_whead = nc.declare_dram_parameter("whead", [D, VL], F16, isOutput=False)
    d_whgsum = nc.declare_dram_parameter("whead_gsum", [1, VL], F16, isOutput=False)
    d_headb = nc.declare_dram_parameter("headb", [128, VL // 128], F32, isOutput=False)
    d_ones_col = nc.declare_dram_parameter("ones_col", [128, 1], F32R, isOutput=False)
    d_ones_row = nc.declare_dram_parameter("ones_row", [1, 128], F32R, isOutput=False)
    d_bfc = nc.declare_dram_parameter("bfc", [L, KF, 128, 1], F32, isOutput=False)
    d_bout8 = nc.declare_dram_parameter("bout8", [L, 128, KD], F32, isOutput=False)
    d_bproj8 = nc.declare_dram_parameter("bproj8", [L, 128, KD], F32, isOutput=False)
    d_logits = nc.declare_dram_parameter("logits", [VL, NP], F32, isOutput=True)

    ar_in, ar_out = {}, {}
    for l in range(L):
        for s in range(2):
            ar_in[l, s] = nc.dram_tensor(f"arin_{l}_{s}", [D, NP], F16)
            ar_out[l, s] = nc.dram_tensor(
                f"arout_{l}_{s}", [D, NP], F16, addr_space="Shared"
            )

    AF = mybir.ActivationFunctionType
    ALU = mybir.AluOpType

    with tile.TileContext(nc) as tc, ExitStack() as ctx:
        res = ctx.enter_context(tc.tile_pool(name="res", bufs=1))
        cst = ctx.enter_context(tc.tile_pool(name="cst", bufs=2))
        a1 = ctx.enter_context(tc.tile_pool(name="a1", bufs=2))
        a2 = ctx.enter_context(tc.tile_pool(name="a2", bufs=2))
        a3 = ctx.enter_context(tc.tile_pool(name="a3", bufs=3))
        wgt = ctx.enter_context(tc.tile_pool(name="wgt", bufs=1))
        psR = ctx.enter_context(tc.tile_pool(name="psR", bufs=2, space="PSUM"))
        psB = ctx.enter_context(tc.tile_pool(name="psB", bufs=2, space="PSUM"))
        psM = ctx.enter_context(tc.tile_pool(name="psM", bufs=3, space="PSUM"))

        sqp = ctx.enter_context(tc.tile_pool(name="sqp", bufs=1))

        ones_col = res.tile([128, 1], F32R, tag="ones_col")
        nc.sync.dma_start(out=ones_col[:], in_=d_ones_col[:])
        ones_col16 = res.tile([128, 1], F16, tag="ones_col16")
        nc.vector.memset(ones_col16[:], 1.0)
        ones_row = res.tile([1, 128], F32R, tag="ones_row")
        nc.sync.dma_start(out=ones_row[:], in_=d_ones_row[:])
        ones_f16 = res.tile([1, NP], F16, tag="ones_f16")
        nc.vector.memset(ones_f16[:], 1.0)
        ident = res.tile([128, 128], F32, tag="ident")
        make_identity(nc, ident[:])
        eps_col = res.tile([1, 1], F32, tag="eps")
        nc.vector.memset(eps_col[:], EPS)

        def layer_norm(xt, corr_tag):
            """Fused-batch LN stats: returns (xs 12x[128,NP] f16 scaled tiles,
            mrs [1,NP] f16 row of -mean*rsigma)."""
            mrs = a1.tile([1, NP], F16, tag="mrs")
            sq_tiles = []
            for k in range(KD):
                sq = sqp.tile([128, NP], F16, tag=f"sq{k}")
                with nc.allow_low_precision("fp16 sumsq"):
                    nc.scalar.activation(sq[:], xt[k][:], AF.Square)
                sq_tiles.append(sq)
            bcs = []
            for s in range(2):
                c0 = s * NCH
                p_sums = psR.tile([1, NCH], F32, tag="row")
                for k in range(KD):
                    nc.tensor.matmul(
                        p_sums[:], ones_col16[:], xt[k][:, c0:c0 + NCH],
                        start=(k == 0), stop=(k == KD - 1),
                    )
                p_sqs = psR.tile([1, NCH], F32, tag="row")
                for k in range(KD):
                    nc.tensor.matmul(
                        p_sqs[:], ones_col16[:], sq_tiles[k][:, c0:c0 + NCH],
                        start=(k == 0), stop=(k == KD - 1),
                    )
                m_row = a1.tile([1, NCH], F32, tag="m_row")
                nc.vector.tensor_scalar(m_row[:], p_sums[:], 1.0 / D, None, ALU.mult)
                ms = a1.tile([1, NCH], F32, tag="ms_row")
                nc.vector.tensor_mul(ms[:], m_row[:], m_row[:])
                var = a1.tile([1, NCH], F32, tag="var_row")
                nc.vector.scalar_tensor_tensor(
                    var[:], p_sqs[:], 1.0 / D, ms[:], ALU.mult, ALU.subtract
                )
                std = a1.tile([1, NCH], F32, tag="std_row")
                nc.scalar.activation(std[:], var[:], AF.Sqrt, bias=eps_col[:])
                rs_row = a1.tile([1, NCH], F32R, tag="rs_row")
                with nc.allow_low_precision("f32r rounding"):
                    nc.vector.reciprocal(rs_row[:], std[:])
                with nc.allow_low_precision("fp16 correction row"):
                    nc.vector.scalar_tensor_tensor(
                        mrs[:, c0:c0 + NCH], m_row[:], -1.0,
                        rs_row[:].bitcast(F32), ALU.mult, ALU.mult,
                    )
                p_bc = psB.tile([128, NCH], F32, tag="bc")
                nc.tensor.matmul(p_bc[:], ones_row[:], rs_row[:], start=True, stop=True)
                bcs.append(p_bc)
            xs_tiles = []
            for k in range(KD):
                xs = sqp.tile([128, NP], F16, name=f"xs{k}", tag=f"xs{k}")
                for s in range(2):
                    c0 = s * NCH
                    with nc.allow_low_precision("fp16 matmul input"):
                        nc.vector.tensor_mul(
                            xs[:, c0:c0 + NCH], xt[k][:, c0:c0 + NCH],
                            bcs[s][:],
                        )
                xs_tiles.append(xs)
            return xs_tiles, mrs

        def partial_out(psum, b8_sb, mi, c0, part, on_act):
            with nc.allow_low_precision("fp16 allreduce payload"):
                if on_act:
                    nc.scalar.activation(
                        part[:, c0:c0 + NCH], psum[:], AF.Identity,
                        bias=b8_sb[:, mi:mi + 1],
                    )
                else:
                    nc.vector.tensor_scalar(
                        part[:, c0:c0 + NCH], psum[:], b8_sb[:, mi:mi + 1],
                        None, ALU.add,
                    )

        def all_reduce(l, s):
            if use_cc:
                nc.gpsimd.collective_compute(
                    "AllReduce", ALU.add,
                    replica_groups=[list(range(NC))],
                    ins=[ar_in[l, s][:].opt()],
                    outs=[ar_out[l, s][:].opt()],
                )
            else:
                nc.gpsimd.dma_start(out=ar_out[l, s][:], in_=ar_in[l, s][:])

        def refresh_xt(xt, l, s):
            # accumulating DMA: xt += ar_out (SWDGE CCE add, no engine work)
            for k in range(KD):
                nc.gpsimd.dma_start(
                    out=xt[k][:], in_=ar_out[l, s][k * 128:(k + 1) * 128, :],
                    accum_op=ALU.add,
                )

        for _rep in range(reps):
            xt = []
            for k in range(KD):
                t = res.tile([128, NP], F16, name=f"xt{k}", tag=f"xt{k}")
                nc.sync.dma_start(
                    out=t[:], in_=d_x0t[k * 128:(k + 1) * 128, :]
                )
                xt.append(t)

            for l in range(L):
                bfc_t = []
                for mi in range(KF):
                    bt = cst.tile([128, 1], F32, name=f"bfc{mi}", tag=f"bfc{mi}")
                    nc.sync.dma_start(out=bt[:], in_=d_bfc[l, mi])
                    bfc_t.append(bt)
                bout8_sb = cst.tile([128, KD], F32, tag="bout8")
                nc.sync.dma_start(out=bout8_sb[:], in_=d_bout8[l])
                bproj8_sb = cst.tile([128, KD], F32, tag="bproj8")
                nc.sync.dma_start(out=bproj8_sb[:], in_=d_bproj8[l])

                wa = []
                for k in range(KD):
                    t = wgt.tile([128, 256], F16, tag=f"wa{k}")
                    nc.sync.dma_start(
                        out=t[:], in_=d_wattn[l, k * 128:(k + 1) * 128, :]
                    )
                    wa.append(t)
                wagsum = cst.tile([1, 256], F16, tag="wagsum")
                nc.sync.dma_start(out=wagsum[:], in_=d_wagsum[l])
                wabeta = cst.tile([1, 256], F16, tag="wabeta")
                nc.sync.dma_start(out=wabeta[:], in_=d_wabeta[l])
                wp = []
                for j in range(HPC):
                    t = wgt.tile([DH, D], F16, tag=f"wp{j}")
                    nc.sync.dma_start(out=t[:], in_=d_wproj[l, j])
                    wp.append(t)

                # ---------- attention sublayer ----------
                xs, mrs = layer_norm(xt, f"l{l}a")

                whsb = [[None] * 3 for _ in range(2)]
                for s in range(2):
                    for mi, (ms, msz) in enumerate(MTS[s]):
                        p = psM.tile([128, 256], F32, tag="mm")
                        for k in range(KD):
                            nc.tensor.matmul(
                                p[:msz, :], xs[k][:, ms:ms + msz], wa[k][:],
                                start=(k == 0), stop=False,
                            )
                        nc.tensor.matmul(
                            p[:msz, :], mrs[:, ms:ms + msz], wagsum[:],
                            start=False, stop=False,
                        )
                        nc.tensor.matmul(
                            p[:msz, :], ones_f16[:, ms:ms + msz], wabeta[:],
                            start=False, stop=True,
                        )
                        w = a1.tile([128, 198], F32R, tag=f"whsb{mi}_{s}")
                        nc.scalar.copy(w[:msz, 0:196], p[:msz, 0:196])
                        nc.vector.tensor_scalar(
                            w[:msz, 196:198], p[:msz, 192:194], 0.2, None, ALU.mult
                        )
                        whsb[s][mi] = w

                aggt = [
                    a1.tile([DH, NP], F16, name=f"aggt{j}", tag=f"aggt{j}")
                    for j in range(HPC)
                ]
                for s in range(2):
                    erow = [
                        a1.tile([1, NCH], F32R, name=f"er{j}_{s}", tag=f"er{j}_{s}")
                        for j in range(HPC)
                    ]
                    for mi, (ms, msz) in enumerate(MTS[s]):
                        for j in range(HPC):
                            pt = psR.tile([1, 128], F32, tag="row")
                            nc.tensor.transpose(
                                pt[:, :msz],
                                whsb[s][mi][:msz, 194 + j:195 + j].bitcast(F32),
                                ident[:msz, :msz],
                            )
                            lo = ms - s * NCH
                            nc.scalar.copy(erow[j][:, lo:lo + msz], pt[:, :msz])

                    for j in range(HPC):
                        p_er = psB.tile([128, NCH], F32, tag="bc")
                        nc.tensor.matmul(
                            p_er[:], ones_row[:], erow[j][:], start=True, stop=True
                        )
                        e_tiles = []
                        for mi in range(3):
                            rsz = MT_REAL[mi]
                            e1 = a2.tile([128, NCH], F32R, tag=f"e{mi}")
                            nc.scalar.activation(
                                e1[:rsz, :], p_er[:rsz, :], AF.Exp,
                                bias=whsb[s][mi][:rsz, 192 + j:193 + j].bitcast(F32),
                            )
                            e2 = a2.tile([128, NCH], F32, tag="e2")
                            nc.scalar.activation(
                                e2[:rsz, :], p_er[:rsz, :], AF.Exp, scale=0.2,
                                bias=whsb[s][mi][:rsz, 196 + j:197 + j].bitcast(F32),
                            )
                            nc.vector.tensor_max(
                                e1[:rsz, :], e1[:rsz, :].bitcast(F32), e2[:rsz, :]
                            )
                            e_tiles.append(e1)
                        p_s = psR.tile([1, NCH], F32, tag="row")
                        for mi in range(3):
                            rsz = MT_REAL[mi]
                            nc.tensor.matmul(
                                p_s[:], ones_col[:rsz, :], e_tiles[mi][:rsz, :],
                                start=(mi == 0), stop=(mi == 2),
                            )
                        r_row = a1.tile([1, NCH], F32R, tag="r_row")
                        with nc.allow_low_precision("f32r rounding"):
                            nc.vector.reciprocal(r_row[:], p_s[:])
                        p_rb2 = psB.tile([DH, NCH], F32, tag="bc")
                        nc.tensor.matmul(
                            p_rb2[:], ones_row[:, :DH], r_row[:], start=True, stop=True
                        )
                        rb_sb = a1.tile([DH, NCH], F32, tag="rb_sb")
                        nc.scalar.copy(rb_sb[:], p_rb2[:])
                        p_agg = psM.tile([DH, NCH], F32, tag="mm")
                        for mi in range(3):
                            rsz = MT_REAL[mi]
                            nc.tensor.matmul(
                                p_agg[:],
                                whsb[s][mi][:rsz, j * DH:(j + 1) * DH],
                                e_tiles[mi][:rsz, :],
                                start=(mi == 0), stop=(mi == 2),
                            )
                        c0 = s * NCH
                        with nc.allow_low_precision("fp16 agg"):
                            nc.vector.tensor_mul(
                                aggt[j][:, c0:c0 + NCH], p_agg[:], rb_sb[:]
                            )

                for mi in range(KD):
                    part = a3.tile([128, NP], F16, tag="part")
                    for s in range(2):
                        c0 = s * NCH
                        p = psM.tile([128, NCH], F32, tag="mm")
                        for j in range(HPC):
                            nc.tensor.matmul(
                                p[:], wp[j][:, mi * 128:(mi + 1) * 128],
                                aggt[j][:, c0:c0 + NCH],
                                start=(j == 0), stop=(j == HPC - 1),
                            )
                        partial_out(p, bproj8_sb, mi, c0, part, on_act=False)
                    nc.scalar.dma_start(
                        out=ar_in[l, 0][mi * 128:(mi + 1) * 128, :], in_=part[:]
                    )
                all_reduce(l, 0)
                refresh_xt(xt, l, 0)

                # ---------- FFN sublayer ----------
                wfc_sb = []
                for k in range(KD):
                    t = wgt.tile([128, VL], F16, tag=f"wbig{k}")
                    nc.sync.dma_start(
                        out=t[:, 0:FFL], in_=d_wfc[l, k * 128:(k + 1) * 128, :]
                    )
                    wfc_sb.append(t)
                wfcgsum = cst.tile([1, FFL], F16, tag="wfcgsum")
                nc.sync.dma_start(out=wfcgsum[:], in_=d_wfcgsum[l])
                wout_sb = []
                for k in range(KF):
                    t = wgt.tile([128, D], F16, tag=f"wo{k}")
                    nc.sync.dma_start(
                        out=t[:], in_=d_wout[l, k * 128:(k + 1) * 128, :]
                    )
                    wout_sb.append(t)

                xs2, mrs2 = layer_norm(xt, f"l{l}f")
                g_tiles = []
                for mi in range(KF):
                    g = a2.tile([128, NP], F16, tag=f"g{mi}")
                    for s in range(2):
                        c0 = s * NCH
                        p = psM.tile([128, NCH], F32, tag="mm")
                        for k in range(KD):
                            nc.tensor.matmul(
                                p[:], wfc_sb[k][:, mi * 128:(mi + 1) * 128],
                                xs2[k][:, c0:c0 + NCH],
                                start=(k == 0), stop=False,
                            )
                        nc.tensor.matmul(
                            p[:], wfcgsum[:, mi * 128:(mi + 1) * 128],
                            mrs2[:, c0:c0 + NCH], start=False, stop=True,
                        )
                        nc.scalar.activation(
                            g[:, c0:c0 + NCH], p[:], AF.Gelu, bias=bfc_t[mi][:]
                        )
                    g_tiles.append(g)
                for mi in range(KD):
                    part = a3.tile([128, NP], F16, tag="part")
                    for s in range(2):
                        c0 = s * NCH
                        p = psM.tile([128, NCH], F32, tag="mm")
                        for k in range(KF):
                            nc.tensor.matmul(
                                p[:], wout_sb[k][:, mi * 128:(mi + 1) * 128],
                                g_tiles[k][:, c0:c0 + NCH],
                                start=(k == 0), stop=(k == KF - 1),
                            )
                        partial_out(p, bout8_sb, mi, c0, part, on_act=True)
                    nc.scalar.dma_start(
                        out=ar_in[l, 1][mi * 128:(mi + 1) * 128, :], in_=part[:]
                    )
                all_reduce(l, 1)
                refresh_xt(xt, l, 1)

            # ---------- final LN + vocab-sharded head ----------
            wh_sb = []
            for k in range(KD):
                t = wgt.tile([128, VL], F16, name=f"whd{k}", tag=f"wbig{k}")
                nc.sync.dma_start(out=t[:], in_=d_whead[k * 128:(k + 1) * 128, :])
                wh_sb.append(t)
            whgsum = cst.tile([1, VL], F16, tag="whgsum")
            nc.sync.dma_start(out=whgsum[:], in_=d_whgsum[:])
            headb_sb = cst.tile([128, VL // 128], F32, tag="headb")
            nc.sync.dma_start(out=headb_sb[:], in_=d_headb[:])

            xsf, mrsf = layer_norm(xt, "lnf")
            for mi in range(VL // 128):
                lg = a3.tile([128, NP], F32, tag="lg")
                for s in range(2):
                    c0 = s * NCH
                    p = psM.tile([128, NCH], F32, tag="mm")
                    for k in range(KD):
                        nc.tensor.matmul(
                            p[:], wh_sb[k][:, mi * 128:(mi + 1) * 128],
                            xsf[k][:, c0:c0 + NCH],
                            start=(k == 0), stop=False,
                        )
                    nc.tensor.matmul(
                        p[:], whgsum[:, mi * 128:(mi + 1) * 128],
                        mrsf[:, c0:c0 + NCH], start=False, stop=True,
                    )
                    nc.scalar.activation(
                        lg[:, c0:c0 + NCH], p[:], AF.Identity,
                        bias=headb_sb[:, mi:mi + 1],
                    )
                nc.scalar.dma_start(
                    out=d_logits[mi * 128:(mi + 1) * 128, :], in_=lg[:]
                )

    nc.compile()
    return nc


def _get_nc(reps=1, use_cc=True, ffn_bf16=True):
    key = f"nc{reps}_{use_cc}_{ffn_bf16}"
    if key not in _CACHE:
        _CACHE[key] = _build_nc(reps, use_cc, ffn_bf16)
    return _CACHE[key]


# --------------------------------------------------------------------------
# numpy fallback (exact reference semantics for arbitrary edges)
# --------------------------------------------------------------------------

def _numpy_forward(inp):
    from scipy.special import erf

    def ln(x, g, b):
        m = x.mean(-1, keepdims=True)
        v = ((x - m) ** 2).mean(-1, keepdims=True)
        return (x - m) / np.sqrt(v + EPS) * g + b

    f32 = np.float32
    objs_e = np.asarray(inp["obj_emb_w"])[np.asarray(inp["objs"])]
    pe = np.asarray(inp["poss_emb_w"])[np.asarray(inp["poss"])]
    nfeat = np.concatenate([objs_e, pe[:, :NOBJ], pe[:, NOBJ:]], axis=-1)
    z = np.asarray(inp["tok_emb"])[np.asarray(inp["z_indices"])]
    x = np.concatenate([nfeat, z], axis=1) + np.asarray(inp["pos_emb"])[:, :T]
    x = x.reshape(N, D).astype(f32)
    src = np.asarray(inp["src"]).astype(np.int64)
    dst = np.asarray(inp["dst"]).astype(np.int64)
    for l in range(L):
        h = ln(x, inp["ln1_g"][l], inp["ln1_b"][l])
        Wh = (h @ np.asarray(inp["W_attn"][l])).reshape(N, H, DH)
        el = np.einsum("nhd,hd->nh", Wh, np.asarray(inp["a_l"][l]))
        er = np.einsum("nhd,hd->nh", Wh, np.asarray(inp["a_r"][l]))
        e = el[src] + er[dst]
        e = np.where(e >= 0, e, 0.2 * e)
        m = np.full((N, H), -np.inf, f32)
        np.maximum.at(m, dst, e)
        m[~np.isfinite(m)] = 0.0
        ex = np.exp(e - m[dst])
        s = np.zeros((N, H), f32)
        np.add.at(s, dst, ex)
        alpha = ex / s[dst]
        agg = np.zeros((N, H, DH), f32)
        np.add.at(agg, dst, alpha[:, :, None] * Wh[src])
        x = x + agg.reshape(N, D) @ np.asarray(inp["W_proj"][l]) \
            + np.asarray(inp["b_proj"][l])
        h2 = ln(x, inp["ln2_g"][l], inp["ln2_b"][l])
        ff = h2 @ np.asarray(inp["W_fc"][l]) + np.asarray(inp["b_fc"][l])
        ff = ff * 0.5 * (1.0 + erf(ff / np.sqrt(2.0)))
        x = x + ff @ np.asarray(inp["W_out"][l]) + np.asarray(inp["b_out"][l])
    x = ln(x, inp["lnf_g"], inp["lnf_b"])
    return (x @ np.asarray(inp["head_w"])).reshape(B, T, V).astype(f32)


# --------------------------------------------------------------------------
# public entry
# --------------------------------------------------------------------------

def _edges_are_block_diag(inp):
    src, dst = _block_diag_edges_np()
    s = np.asarray(inp["src"])
    d = np.asarray(inp["dst"])
    return (
        s.shape == src.shape
        and np.array_equal(s.astype(np.int64), src)
        and np.array_equal(d.astype(np.int64), dst)
    )


def _assemble(results):
    full = np.concatenate([results[c]["logits"] for c in range(NC)], axis=0)
    out = np.empty((N, V), np.float32)
    for b in range(B):
        out[b * T:(b + 1) * T] = full[:, b * NCH:b * NCH + T].T
    return out.reshape(B, T, V)


def kernel(**inputs):
    if not _edges_are_block_diag(inputs):
        return _numpy_forward(inputs)
    from concourse import bass2jax

    in_maps = _host_inputs(inputs)
    results = bass2jax.run_bass_via_pjrt(_get_nc(), in_maps, n_cores=NC)
    return _assemble(results)


# --------------------------------------------------------------------------
# benchmarking (repeated execution, device-resident inputs)
# --------------------------------------------------------------------------

def _make_runner(nc):
    """Persistent jitted shard_map callable for nc (multi-core), mirroring
    bass2jax.run_bass_via_pjrt but reusable across calls."""
    import jax
    from jax.sharding import Mesh, PartitionSpec
    from jax.experimental.shard_map import shard_map
    from concourse import bass2jax, mybir as _mybir

    bass2jax.install_neuronx_cc_hook()
    partition_name = nc.partition_id_tensor.name if nc.partition_id_tensor else None
    in_names, out_names, out_avals, zero_outs = [], [], [], []
    for alloc in nc.m.functions[0].allocations:
        if not isinstance(alloc, _mybir.MemoryLocationSet):
            continue
        name = alloc.memorylocations[0].name
        if alloc.kind == "ExternalInput":
            if name != partition_name:
                in_names.append(name)
        elif alloc.kind == "ExternalOutput":
            shape = tuple(alloc.tensor_shape)
            dtype = _mybir.dt.np(alloc.dtype)
            out_names.append(name)
            out_avals.append(jax.core.ShapedArray(shape, dtype))
            zero_outs.append(np.zeros(shape, dtype))
    n_params = len(in_names)
    all_in_names = list(in_names) + list(out_names)
    if partition_name is not None:
        all_in_names.append(partition_name)

    def _body(*args):
        operands = list(args)
        if partition_name is not None:
            operands.append(bass2jax.partition_id_tensor())
        return tuple(
            bass2jax._bass_exec_p.bind(
                *operands,
                out_avals=tuple(out_avals),
                in_names=tuple(all_in_names),
                out_names=tuple(out_names),
                lowering_input_output_aliases=(),
                sim_require_finite=True,
                sim_require_nnan=True,
                nc=nc,
            )
        )

    devices = jax.devices()[:NC]
    mesh = Mesh(np.asarray(devices), ("core",))
    n_outs = len(out_names)
    in_specs = (PartitionSpec("core"),) * (n_params + n_outs)
    out_specs = (PartitionSpec("core"),) * n_outs
    donate = tuple(range(n_params, n_params + n_outs))
    fn = jax.jit(
        shard_map(_body, mesh=mesh, in_specs=in_specs, out_specs=out_specs,
                  check_rep=False),
        donate_argnums=donate, keep_unused=True,
    )
    return fn, in_names, out_names, zero_outs, mesh


def _timed_run(nc, in_maps, iters):
    """Median wall time (s) per execution with device-resident inputs."""
    import jax

    from jax.sharding import NamedSharding, PartitionSpec

    fn, in_names, out_names, zero_outs, mesh = _make_runner(nc)
    shard = NamedSharding(mesh, PartitionSpec("core"))
    concat_in = [
        np.concatenate([np.asarray(m[name]) for m in in_maps], axis=0)
        for name in in_names
    ]
    dev_in = [jax.device_put(a, shard) for a in concat_in]
    jax.block_until_ready(dev_in)

    def zeros():
        zs = [
            jax.device_put(
                np.zeros((NC * z.shape[0], *z.shape[1:]), z.dtype), shard
            )
            for z in zero_outs
        ]
        jax.block_until_ready(zs)
        return zs

    outs = fn(*dev_in, *zeros())  # warm-up/compile
    jax.block_until_ready(outs)
    times = []
    for _ in range(iters):
        zs = zeros()
        t0 = time.perf_counter()
        outs = fn(*dev_in, *zs)
        jax.block_until_ready(outs)
        times.append(time.perf_counter() - t0)
    return float(np.min(times)), outs, out_names


def bench(inputs, iters=16):
    """HW ns per network pass via reps-differential (cancels dispatch cost)."""
    in_maps = _host_inputs(inputs)
    t1, _, _ = _timed_run(_get_nc(1), in_maps, iters)
    t9, _, _ = _timed_run(_get_nc(9), in_maps, iters)
    print(f"  wall/iter reps1: {t1 * 1e6:.0f} us,  reps9: {t9 * 1e6:.0f} us")
    return max(t9 - t1, 0.0) / 8 * 1e9


# revision 25
# speedup vs baseline: 1.0396x; 1.0396x over previous
"""Trainium2 Bass kernel for nn_GAT_42786464203341.

8-way tensor parallel (Megatron-style) over one trn2 chip:
  - GAT edges are block-diagonal fully-connected per sample, so message
    passing is dense per-sample attention with scores leaky(el[i] + er[j]),
    softmaxed over source i (exp needs no max-subtraction, scores O(1)).
  - Activations feature-major (x^T: [D, nodes]); both samples fused on the
    free axis ([128, 532] SBUF tiles, per-266 psum chunks).
  - LayerNorm gains are folded into the weights host-side; the mean term and
    LN bias enter each consumer GEMM as K=1 rank-1 correction matmuls, so LN
    application is a single fused scale-and-cast (x * rsigma -> fp16) per
    k-tile instead of mul+sub+affine.
  - All weights fp16 (full PE rate, FWL weight loads, half the HBM bytes).
    Attention inner math stays f32r.  Matmul accumulation is fp32 PSUM.
  - Attention head-parallel (2 heads/core); W_proj row-sharded -> partial
    [D, nodes] -> AllReduce (fp16).  FFN column/row sharded -> AllReduce.
    Head vocab-sharded; host concatenates the 8 logits slices.
"""

import time
from contextlib import ExitStack

import ml_dtypes
import numpy as np

import concourse.bass as bass
import concourse.tile as tile
from concourse import bacc, mybir
from concourse.masks import make_identity

F32 = mybir.dt.float32
F32R = mybir.dt.float32r
F16 = mybir.dt.float16

B, T, NOBJ = 2, 265, 9
D, H, DH = 1536, 16, 96
V, PV, L, FF = 8192, 512, 3, 6144
N = B * T          # 530
NC = 8             # cores
HPC = H // NC      # heads per core
FFL = FF // NC     # 768
VL = V // NC       # 1024
NCH = T + 1        # 266 (col 265 of each chunk is zero padding)
NP = B * NCH       # 532
KD = D // 128      # 12
KF = FFL // 128    # 6
MTS = [[(0, 128), (128, 128), (256, 10)],
       [(266, 128), (394, 128), (522, 10)]]    # node tiles (fused offsets)
MT_REAL = [128, 128, 9]                        # non-pad rows per node tile
EPS = 1e-5

_CACHE = {}


# --------------------------------------------------------------------------
# host-side input prep
# --------------------------------------------------------------------------

def _block_diag_edges_np():
    base = np.arange(T)
    src = np.concatenate([g * T + np.repeat(base, T) for g in range(B)])
    dst = np.concatenate([g * T + np.tile(base, T) for g in range(B)])
    return src.astype(np.int64), dst.astype(np.int64)


def _host_inputs(inp, ffn_bf16=True):
    f32, f16 = np.float32, np.float16
    objs_e = np.asarray(inp["obj_emb_w"])[np.asarray(inp["objs"])]
    pe = np.asarray(inp["poss_emb_w"])[np.asarray(inp["poss"])]
    nfeat = np.concatenate([objs_e, pe[:, :NOBJ], pe[:, NOBJ:]], axis=-1)
    z = np.asarray(inp["tok_emb"])[np.asarray(inp["z_indices"])]
    x0 = np.concatenate([nfeat, z], axis=1) + np.asarray(inp["pos_emb"])[:, :T]
    x0 = x0.reshape(N, D).astype(f32)

    x0t = np.zeros((D, NP), f16)
    for b in range(B):
        x0t[:, b * NCH:b * NCH + T] = x0[b * T:(b + 1) * T].T.astype(f16)

    W_attn = np.asarray(inp["W_attn"], f32)
    a_l = np.asarray(inp["a_l"], f32)
    a_r = np.asarray(inp["a_r"], f32)
    W_proj = np.asarray(inp["W_proj"], f32)
    W_fc = np.asarray(inp["W_fc"], f32)
    W_out = np.asarray(inp["W_out"], f32)
    head_w = np.asarray(inp["head_w"], f32)
    g1 = np.asarray(inp["ln1_g"], f32)      # [L, D]
    b1 = np.asarray(inp["ln1_b"], f32)
    g2 = np.asarray(inp["ln2_g"], f32)
    b2 = np.asarray(inp["ln2_b"], f32)
    gf = np.asarray(inp["lnf_g"], f32)      # [D]
    bf = np.asarray(inp["lnf_b"], f32)

    def cols(vec, k_tiles):  # [3, D'] -> [3, 128, k_tiles]
        v = np.asarray(vec, f32)
        return np.transpose(v.reshape(3, k_tiles, 128), (0, 2, 1)).copy()

    bout8 = cols(np.asarray(inp["b_out"], f32) / NC, KD)
    bproj8 = cols(np.asarray(inp["b_proj"], f32) / NC, KD)

    maps = []
    for c in range(NC):
        h0 = c * HPC
        wattn = np.zeros((L, D, 256), f32)
        for j in range(HPC):
            hg = h0 + j
            blk = W_attn[:, :, hg * DH:(hg + 1) * DH]         # [3, D, DH]
            wattn[:, :, j * DH:(j + 1) * DH] = blk
            # el/er are linear in h: fold (W_attn-block @ a) into one column
            wattn[:, :, 192 + j] = np.matmul(blk, a_l[:, hg, :, None])[..., 0]
            wattn[:, :, 194 + j] = np.matmul(blk, a_r[:, hg, :, None])[..., 0]
        # LN1 fold: gain into weights; Gsum/Beta correction rows
        wattn_g = wattn * g1[:, :, None]                      # [L, D, 256]
        wa_gsum = np.einsum("ld,ldc->lc", g1, wattn)          # [L, 256]
        wa_beta = np.einsum("ld,ldc->lc", b1, wattn)          # [L, 256]

        wproj = np.stack(
            [W_proj[:, (h0 + j) * DH:(h0 + j + 1) * DH, :] for j in range(HPC)],
            axis=1,
        )
        wfc_sl = W_fc[:, :, c * FFL:(c + 1) * FFL]            # [L, D, FFL]
        wfc_g = wfc_sl * g2[:, :, None]
        wfc_gsum = np.einsum("ld,ldm->lm", g2, wfc_sl)        # [L, FFL]
        # LN2 beta folds into the fc bias column
        bfc_eff = np.asarray(inp["b_fc"], f32)[:, c * FFL:(c + 1) * FFL] \
            + np.einsum("ld,ldm->lm", b2, wfc_sl)             # [L, FFL]
        bfc_cols = np.transpose(bfc_eff.reshape(L, KF, 128), (0, 2, 1)).copy()

        whead_sl = head_w[:, c * VL:(c + 1) * VL]             # [D, VL]
        whead_g = whead_sl * gf[:, None]
        whead_gsum = (gf @ whead_sl)[None, :]                 # [1, VL]
        headb = (bf @ whead_sl).reshape(VL // 128, 128).T.copy()  # [128, 8]

        maps.append({
            "x0t": x0t,
            "wattn": wattn_g.astype(f16),
            "wa_gsum": wa_gsum[:, None, :].astype(f16),       # [L, 1, 256]
            "wa_beta": wa_beta[:, None, :].astype(f16),
            "wproj": np.ascontiguousarray(wproj).astype(f16),
            "wfc": np.ascontiguousarray(wfc_g).astype(f16),
            "wfc_gsum": wfc_gsum[:, None, :].astype(f16),     # [L, 1, FFL]
            "wout": np.ascontiguousarray(W_out[:, c * FFL:(c + 1) * FFL, :]).astype(f16),
            "whead": np.ascontiguousarray(whead_g).astype(f16),
            "whead_gsum": whead_gsum.astype(f16),             # [1, VL]
            "headb": headb,                                   # [128, 8] f32
            "ones_col": np.ones((128, 1), f32),
            "ones_row": np.ones((1, 128), f32),
            "bfc": np.ascontiguousarray(bfc_cols[..., None]), # [L, 128, KF, 1]->?
            "bout8": bout8, "bproj8": bproj8,
        })
    # fix bfc shape: want [L, KF, 128, 1]
    for m in maps:
        b = m["bfc"][..., 0]                                  # [L, 128, KF]
        m["bfc"] = np.ascontiguousarray(np.transpose(b, (0, 2, 1))[..., None])
    return maps


# --------------------------------------------------------------------------
# device program
# --------------------------------------------------------------------------

def _build_nc(reps=1, use_cc=True, ffn_bf16=True):
    nc = bacc.Bacc("TRN2", target_bir_lowering=False, debug=False, num_devices=NC)

    d_x0t = nc.declare_dram_parameter("x0t", [D, NP], F16, isOutput=False)
    d_wattn = nc.declare_dram_parameter("wattn", [L, D, 256], F16, isOutput=False)
    d_wagsum = nc.declare_dram_parameter("wa_gsum", [L, 1, 256], F16, isOutput=False)
    d_wabeta = nc.declare_dram_parameter("wa_beta", [L, 1, 256], F16, isOutput=False)
    d_wproj = nc.declare_dram_parameter("wproj", [L, HPC, DH, D], F16, isOutput=False)
    d_wfc = nc.declare_dram_parameter("wfc", [L, D, FFL], F16, isOutput=False)
    d_wfcgsum = nc.declare_dram_parameter("wfc_gsum", [L, 1, FFL], F16, isOutput=False)
    d_wout = nc.declare_dram_parameter("wout", [L, FFL, D], F16, isOutput=False)
    d_whead = nc.declare_dram_parameter("whead", [D, VL], F16, isOutput=False)
    d_whgsum = nc.declare_dram_parameter("whead_gsum", [1, VL], F16, isOutput=False)
    d_headb = nc.declare_dram_parameter("headb", [128, VL // 128], F32, isOutput=False)
    d_ones_col = nc.declare_dram_parameter("ones_col", [128, 1], F32R, isOutput=False)
    d_ones_row = nc.declare_dram_parameter("ones_row", [1, 128], F32R, isOutput=False)
    d_bfc = nc.declare_dram_parameter("bfc", [L, KF, 128, 1], F32, isOutput=False)
    d_bout8 = nc.declare_dram_parameter("bout8", [L, 128, KD], F32, isOutput=False)
    d_bproj8 = nc.declare_dram_parameter("bproj8", [L, 128, KD], F32, isOutput=False)
    d_logits = nc.declare_dram_parameter("logits", [VL, NP], F32, isOutput=True)

    ar_in, ar_out = {}, {}
    for l in range(L):
        for s in range(2):
            ar_in[l, s] = nc.dram_tensor(f"arin_{l}_{s}", [D, NP], F16)
            ar_out[l, s] = nc.dram_tensor(
                f"arout_{l}_{s}", [D, NP], F16, addr_space="Shared"
            )

    AF = mybir.ActivationFunctionType
    ALU = mybir.AluOpType

    with tile.TileContext(nc) as tc, ExitStack() as ctx:
        res = ctx.enter_context(tc.tile_pool(name="res", bufs=1))
        cst = ctx.enter_context(tc.tile_pool(name="cst", bufs=2))
        a1 = ctx.enter_context(tc.tile_pool(name="a1", bufs=2))
        a2 = ctx.enter_context(tc.tile_pool(name="a2", bufs=2))
        a3 = ctx.enter_context(tc.tile_pool(name="a3", bufs=3))
        wgt = ctx.enter_context(tc.tile_pool(name="wgt", bufs=1))
        psR = ctx.enter_context(tc.tile_pool(name="psR", bufs=2, space="PSUM"))
        psB = ctx.enter_context(tc.tile_pool(name="psB", bufs=2, space="PSUM"))
        psM = ctx.enter_context(tc.tile_pool(name="psM", bufs=3, space="PSUM"))

        sqp = ctx.enter_context(tc.tile_pool(name="sqp", bufs=1))

        ones_col = res.tile([128, 1], F32R, tag="ones_col")
        nc.sync.dma_start(out=ones_col[:], in_=d_ones_col[:])
        ones_col16 = res.tile([128, 1], F16, tag="ones_col16")
        nc.vector.memset(ones_col16[:], 1.0)
        ones_row = res.tile([1, 128], F32R, tag="ones_row")
        nc.sync.dma_start(out=ones_row[:], in_=d_ones_row[:])
        ones_f16 = res.tile([1, NP], F16, tag="ones_f16")
        nc.vector.memset(ones_f16[:], 1.0)
        ident = res.tile([128, 128], F32, tag="ident")
        make_identity(nc, ident[:])
        eps_col = res.tile([1, 1], F32, tag="eps")
        nc.vector.memset(eps_col[:], EPS)

        def layer_norm(xt, corr_tag):
            """Fused-batch LN stats: returns (xs 12x[128,NP] f16 scaled tiles,
            mrs [1,NP] f16 row of -mean*rsigma)."""
            mrs = a1.tile([1, NP], F16, tag="mrs")
            sq_tiles = []
            for k in range(KD):
                sq = sqp.tile([128, NP], F16, tag=f"sq{k}")
                with nc.allow_low_precision("fp16 sumsq"):
                    nc.scalar.activation(sq[:], xt[k][:], AF.Square)
                sq_tiles.append(sq)
            bcs = []
            for s in range(2):
                c0 = s * NCH
                p_sums = psR.tile([1, NCH], F32, tag="row")
                for k in range(KD):
                    nc.tensor.matmul(
                        p_sums[:], ones_col16[:], xt[k][:, c0:c0 + NCH],
                        start=(k == 0), stop=(k == KD - 1),
                    )
                p_sqs = psR.tile([1, NCH], F32, tag="row")
                for k in range(KD):
                    nc.tensor.matmul(
                        p_sqs[:], ones_col16[:], sq_tiles[k][:, c0:c0 + NCH],
                        start=(k == 0), stop=(k == KD - 1),
                    )
                m_row = a1.tile([1, NCH], F32, tag="m_row")
                nc.vector.tensor_scalar(m_row[:], p_sums[:], 1.0 / D, None, ALU.mult)
                ms = a1.tile([1, NCH], F32, tag="ms_row")
                nc.vector.tensor_mul(ms[:], m_row[:], m_row[:])
                var = a1.tile([1, NCH], F32, tag="var_row")
                nc.vector.scalar_tensor_tensor(
                    var[:], p_sqs[:], 1.0 / D, ms[:], ALU.mult, ALU.subtract
                )
                std = a1.tile([1, NCH], F32, tag="std_row")
                nc.scalar.activation(std[:], var[:], AF.Sqrt, bias=eps_col[:])
                rs_row = a1.tile([1, NCH], F32R, tag="rs_row")
                with nc.allow_low_precision("f32r rounding"):
                    nc.vector.reciprocal(rs_row[:], std[:])
                with nc.allow_low_precision("fp16 correction row"):
                    nc.vector.scalar_tensor_tensor(
                        mrs[:, c0:c0 + NCH], m_row[:], -1.0,
                        rs_row[:].bitcast(F32), ALU.mult, ALU.mult,
                    )
                p_bc = psB.tile([128, NCH], F32, tag="bc")
                nc.tensor.matmul(p_bc[:], ones_row[:], rs_row[:], start=True, stop=True)
                bcs.append(p_bc)
            xs_tiles = []
            for k in range(KD):
                xs = sqp.tile([128, NP], F16, name=f"xs{k}", tag=f"xs{k}")
                for s in range(2):
                    c0 = s * NCH
                    with nc.allow_low_precision("fp16 matmul input"):
                        nc.vector.tensor_mul(
                            xs[:, c0:c0 + NCH], xt[k][:, c0:c0 + NCH],
                            bcs[s][:],
                        )
                xs_tiles.append(xs)
            return xs_tiles, mrs

        def partial_out(psum, b8_sb, mi, c0, part, on_act):
            with nc.allow_low_precision("fp16 allreduce payload"):
                if on_act:
                    nc.scalar.activation(
                        part[:, c0:c0 + NCH], psum[:], AF.Identity,
                        bias=b8_sb[:, mi:mi + 1],
                    )
                else:
                    nc.vector.tensor_scalar(
                        part[:, c0:c0 + NCH], psum[:], b8_sb[:, mi:mi + 1],
                        None, ALU.add,
                    )

        def all_reduce(l, s):
            if use_cc:
                nc.gpsimd.collective_compute(
                    "AllReduce", ALU.add,
                    replica_groups=[list(range(NC))],
                    ins=[ar_in[l, s][:].opt()],
                    outs=[ar_out[l, s][:].opt()],
                )
            else:
                nc.gpsimd.dma_start(out=ar_out[l, s][:], in_=ar_in[l, s][:])

        def refresh_xt(xt, l, s):
            for k in range(KD):
                tmp = a3.tile([128, NP], F16, tag="artmp")
                nc.gpsimd.dma_start(
                    out=tmp[:], in_=ar_out[l, s][k * 128:(k + 1) * 128, :]
                )
                with nc.allow_low_precision("fp16 residual"):
                    nc.vector.tensor_add(xt[k][:], xt[k][:], tmp[:])

        for _rep in range(reps):
            xt = []
            for k in range(KD):
                t = res.tile([128, NP], F16, name=f"xt{k}", tag=f"xt{k}")
                nc.sync.dma_start(
                    out=t[:], in_=d_x0t[k * 128:(k + 1) * 128, :]
                )
                xt.append(t)

            for l in range(L):
                bfc_t = []
                for mi in range(KF):
                    bt = cst.tile([128, 1], F32, name=f"bfc{mi}", tag=f"bfc{mi}")
                    nc.sync.dma_start(out=bt[:], in_=d_bfc[l, mi])
                    bfc_t.append(bt)
                bout8_sb = cst.tile([128, KD], F32, tag="bout8")
                nc.sync.dma_start(out=bout8_sb[:], in_=d_bout8[l])
                bproj8_sb = cst.tile([128, KD], F32, tag="bproj8")
                nc.sync.dma_start(out=bproj8_sb[:], in_=d_bproj8[l])

                wa = []
                for k in range(KD):
                    t = wgt.tile([128, 256], F16, tag=f"wa{k}")
                    nc.sync.dma_start(
                        out=t[:], in_=d_wattn[l, k * 128:(k + 1) * 128, :]
                    )
                    wa.append(t)
                wagsum = cst.tile([1, 256], F16, tag="wagsum")
                nc.sync.dma_start(out=wagsum[:], in_=d_wagsum[l])
                wabeta = cst.tile([1, 256], F16, tag="wabeta")
                nc.sync.dma_start(out=wabeta[:], in_=d_wabeta[l])
                wp = []
                for j in range(HPC):
                    t = wgt.tile([DH, D], F16, tag=f"wp{j}")
                    nc.sync.dma_start(out=t[:], in_=d_wproj[l, j])
                    wp.append(t)

                # ---------- attention sublayer ----------
                xs, mrs = layer_norm(xt, f"l{l}a")

                whsb = [[None] * 3 for _ in range(2)]
                for s in range(2):
                    for mi, (ms, msz) in enumerate(MTS[s]):
                        p = psM.tile([128, 256], F32, tag="mm")
                        for k in range(KD):
                            nc.tensor.matmul(
                                p[:msz, :], xs[k][:, ms:ms + msz], wa[k][:],
                                start=(k == 0), stop=False,
                            )
                        nc.tensor.matmul(
                            p[:msz, :], mrs[:, ms:ms + msz], wagsum[:],
                            start=False, stop=False,
                        )
                        nc.tensor.matmul(
                            p[:msz, :], ones_f16[:, ms:ms + msz], wabeta[:],
                            start=False, stop=True,
                        )
                        w = a1.tile([128, 198], F32R, tag=f"whsb{mi}_{s}")
                        nc.scalar.copy(w[:msz, 0:196], p[:msz, 0:196])
                        nc.vector.tensor_scalar(
                            w[:msz, 196:198], p[:msz, 192:194], 0.2, None, ALU.mult
                        )
                        whsb[s][mi] = w

                aggt = [
                    a1.tile([DH, NP], F16, name=f"aggt{j}", tag=f"aggt{j}")
                    for j in range(HPC)
                ]
                for s in range(2):
                    erow = [
                        a1.tile([1, NCH], F32R, name=f"er{j}_{s}", tag=f"er{j}_{s}")
                        for j in range(HPC)
                    ]
                    for mi, (ms, msz) in enumerate(MTS[s]):
                        for j in range(HPC):
                            pt = psR.tile([1, 128], F32, tag="row")
                            nc.tensor.transpose(
                                pt[:, :msz],
                                whsb[s][mi][:msz, 194 + j:195 + j].bitcast(F32),
                                ident[:msz, :msz],
                            )
                            lo = ms - s * NCH
                            nc.scalar.copy(erow[j][:, lo:lo + msz], pt[:, :msz])

                    for j in range(HPC):
                        p_er = psB.tile([128, NCH], F32, tag="bc")
                        nc.tensor.matmul(
                            p_er[:], ones_row[:], erow[j][:], start=True, stop=True
                        )
                        e_tiles = []
                        for mi in range(3):
                            rsz = MT_REAL[mi]
                            e1 = a2.tile([128, NCH], F32R, tag=f"e{mi}")
                            nc.scalar.activation(
                                e1[:rsz, :], p_er[:rsz, :], AF.Exp,
                                bias=whsb[s][mi][:rsz, 192 + j:193 + j].bitcast(F32),
                            )
                            e2 = a2.tile([128, NCH], F32, tag="e2")
                            nc.scalar.activation(
                                e2[:rsz, :], p_er[:rsz, :], AF.Exp, scale=0.2,
                                bias=whsb[s][mi][:rsz, 196 + j:197 + j].bitcast(F32),
                            )
                            nc.vector.tensor_max(
                                e1[:rsz, :], e1[:rsz, :].bitcast(F32), e2[:rsz, :]
                            )
                            e_tiles.append(e1)
                        p_s = psR.tile([1, NCH], F32, tag="row")
                        for mi in range(3):
                            rsz = MT_REAL[mi]
                            nc.tensor.matmul(
                                p_s[:], ones_col[:rsz, :], e_tiles[mi][:rsz, :],
                                start=(mi == 0), stop=(mi == 2),
                            )
                        r_row = a1.tile([1, NCH], F32R, tag="r_row")
                        with nc.allow_low_precision("f32r rounding"):
                            nc.vector.reciprocal(r_row[:], p_s[:])
                        p_rb2 = psB.tile([DH, NCH], F32, tag="bc")
                        nc.tensor.matmul(
                            p_rb2[:], ones_row[:, :DH], r_row[:], start=True, stop=True
                        )
                        rb_sb = a1.tile([DH, NCH], F32, tag="rb_sb")
                        nc.scalar.copy(rb_sb[:], p_rb2[:])
                        p_agg = psM.tile([DH, NCH], F32, tag="mm")
                        for mi in range(3):
                            rsz = MT_REAL[mi]
                            nc.tensor.matmul(
                                p_agg[:],
                                whsb[s][mi][:rsz, j * DH:(j + 1) * DH],
                                e_tiles[mi][:rsz, :],
                                start=(mi == 0), stop=(mi == 2),
                            )
                        c0 = s * NCH
                        with nc.allow_low_precision("fp16 agg"):
                            nc.vector.tensor_mul(
                                aggt[j][:, c0:c0 + NCH], p_agg[:], rb_sb[:]
                            )

                for mi in range(KD):
                    part = a3.tile([128, NP], F16, tag="part")
                    for s in range(2):
                        c0 = s * NCH
                        p = psM.tile([128, NCH], F32, tag="mm")
                        for j in range(HPC):
                            nc.tensor.matmul(
                                p[:], wp[j][:, mi * 128:(mi + 1) * 128],
                                aggt[j][:, c0:c0 + NCH],
                                start=(j == 0), stop=(j == HPC - 1),
                            )
                        partial_out(p, bproj8_sb, mi, c0, part, on_act=False)
                    nc.scalar.dma_start(
                        out=ar_in[l, 0][mi * 128:(mi + 1) * 128, :], in_=part[:]
                    )
                all_reduce(l, 0)
                refresh_xt(xt, l, 0)

                # ---------- FFN sublayer ----------
                wfc_sb = []
                for k in range(KD):
                    t = wgt.tile([128, VL], F16, tag=f"wbig{k}")
                    nc.sync.dma_start(
                        out=t[:, 0:FFL], in_=d_wfc[l, k * 128:(k + 1) * 128, :]
                    )
                    wfc_sb.append(t)
                wfcgsum = cst.tile([1, FFL], F16, tag="wfcgsum")
                nc.sync.dma_start(out=wfcgsum[:], in_=d_wfcgsum[l])
                wout_sb = []
                for k in range(KF):
                    t = wgt.tile([128, D], F16, tag=f"wo{k}")
                    nc.sync.dma_start(
                        out=t[:], in_=d_wout[l, k * 128:(k + 1) * 128, :]
                    )
                    wout_sb.append(t)

                xs2, mrs2 = layer_norm(xt, f"l{l}f")
                g_tiles = []
                for mi in range(KF):
                    g = a2.tile([128, NP], F16, tag=f"g{mi}")
                    for s in range(2):
                        c0 = s * NCH
                        p = psM.tile([128, NCH], F32, tag="mm")
                        for k in range(KD):
                            nc.tensor.matmul(
                                p[:], wfc_sb[k][:, mi * 128:(mi + 1) * 128],
                                xs2[k][:, c0:c0 + NCH],
                                start=(k == 0), stop=False,
                            )
                        nc.tensor.matmul(
                            p[:], wfcgsum[:, mi * 128:(mi + 1) * 128],
                            mrs2[:, c0:c0 + NCH], start=False, stop=True,
                        )
                        nc.scalar.activation(
                            g[:, c0:c0 + NCH], p[:], AF.Gelu, bias=bfc_t[mi][:]
                        )
                    g_tiles.append(g)
                for mi in range(KD):
                    part = a3.tile([128, NP], F16, tag="part")
                    for s in range(2):
                        c0 = s * NCH
                        p = psM.tile([128, NCH], F32, tag="mm")
                        for k in range(KF):
                            nc.tensor.matmul(
                                p[:], wout_sb[k][:, mi * 128:(mi + 1) * 128],
                                g_tiles[k][:, c0:c0 + NCH],
                                start=(k == 0), stop=(k == KF - 1),
                            )
                        partial_out(p, bout8_sb, mi, c0, part, on_act=True)
                    nc.scalar.dma_start(
                        out=ar_in[l, 1][mi * 128:(mi + 1) * 128, :], in_=part[:]
                    )
                all_reduce(l, 1)
                refresh_xt(xt, l, 1)

            # ---------- final LN + vocab-sharded head ----------
            wh_sb = []
            for k in range(KD):
                t = wgt.tile([128, VL], F16, name=f"whd{k}", tag=f"wbig{k}")
                nc.sync.dma_start(out=t[:], in_=d_whead[k * 128:(k + 1) * 128, :])
                wh_sb.append(t)
            whgsum = cst.tile([1, VL], F16, tag="whgsum")
            nc.sync.dma_start(out=whgsum[:], in_=d_whgsum[:])
            headb_sb = cst.tile([128, VL // 128], F32, tag="headb")
            nc.sync.dma_start(out=headb_sb[:], in_=d_headb[:])

            xsf, mrsf = layer_norm(xt, "lnf")
            for mi in range(VL // 128):
                lg = a3.tile([128, NP], F32, tag="lg")
                for s in range(2):
                    c0 = s * NCH
                    p = psM.tile([128, NCH], F32, tag="mm")
                    for k in range(KD):
                        nc.tensor.matmul(
                            p[:], wh_sb[k][:, mi * 128:(mi + 1) * 128],
                            xsf[k][:, c0:c0 + NCH],
                            start=(k == 0), stop=False,
                        )
                    nc.tensor.matmul(
                        p[:], whgsum[:, mi * 128:(mi + 1) * 128],
                        mrsf[:, c0:c0 + NCH], start=False, stop=True,
                    )
                    nc.scalar.activation(
                        lg[:, c0:c0 + NCH], p[:], AF.Identity,
                        bias=headb_sb[:, mi:mi + 1],
                    )
                nc.scalar.dma_start(
                    out=d_logits[mi * 128:(mi + 1) * 128, :], in_=lg[:]
                )

    nc.compile()
    return nc


def _get_nc(reps=1, use_cc=True, ffn_bf16=True):
    key = f"nc{reps}_{use_cc}_{ffn_bf16}"
    if key not in _CACHE:
        _CACHE[key] = _build_nc(reps, use_cc, ffn_bf16)
    return _CACHE[key]


# --------------------------------------------------------------------------
# numpy fallback (exact reference semantics for arbitrary edges)
# --------------------------------------------------------------------------

def _numpy_forward(inp):
    from scipy.special import erf

    def ln(x, g, b):
        m = x.mean(-1, keepdims=True)
        v = ((x - m) ** 2).mean(-1, keepdims=True)
        return (x - m) / np.sqrt(v + EPS) * g + b

    f32 = np.float32
    objs_e = np.asarray(inp["obj_emb_w"])[np.asarray(inp["objs"])]
    pe = np.asarray(inp["poss_emb_w"])[np.asarray(inp["poss"])]
    nfeat = np.concatenate([objs_e, pe[:, :NOBJ], pe[:, NOBJ:]], axis=-1)
    z = np.asarray(inp["tok_emb"])[np.asarray(inp["z_indices"])]
    x = np.concatenate([nfeat, z], axis=1) + np.asarray(inp["pos_emb"])[:, :T]
    x = x.reshape(N, D).astype(f32)
    src = np.asarray(inp["src"]).astype(np.int64)
    dst = np.asarray(inp["dst"]).astype(np.int64)
    for l in range(L):
        h = ln(x, inp["ln1_g"][l], inp["ln1_b"][l])
        Wh = (h @ np.asarray(inp["W_attn"][l])).reshape(N, H, DH)
        el = np.einsum("nhd,hd->nh", Wh, np.asarray(inp["a_l"][l]))
        er = np.einsum("nhd,hd->nh", Wh, np.asarray(inp["a_r"][l]))
        e = el[src] + er[dst]
        e = np.where(e >= 0, e, 0.2 * e)
        m = np.full((N, H), -np.inf, f32)
        np.maximum.at(m, dst, e)
        m[~np.isfinite(m)] = 0.0
        ex = np.exp(e - m[dst])
        s = np.zeros((N, H), f32)
        np.add.at(s, dst, ex)
        alpha = ex / s[dst]
        agg = np.zeros((N, H, DH), f32)
        np.add.at(agg, dst, alpha[:, :, None] * Wh[src])
        x = x + agg.reshape(N, D) @ np.asarray(inp["W_proj"][l]) \
            + np.asarray(inp["b_proj"][l])
        h2 = ln(x, inp["ln2_g"][l], inp["ln2_b"][l])
        ff = h2 @ np.asarray(inp["W_fc"][l]) + np.asarray(inp["b_fc"][l])
        ff = ff * 0.5 * (1.0 + erf(ff / np.sqrt(2.0)))
        x = x + ff @ np.asarray(inp["W_out"][l]) + np.asarray(inp["b_out"][l])
    x = ln(x, inp["lnf_g"], inp["lnf_b"])
    return (x @ np.asarray(inp["head_w"])).reshape(B, T, V).astype(f32)


# --------------------------------------------------------------------------
# public entry
# --------------------------------------------------------------------------

def _edges_are_block_diag(inp):
    src, dst = _block_diag_edges_np()
    s = np.asarray(inp["src"])
    d = np.asarray(inp["dst"])
    return (
        s.shape == src.shape
        and np.array_equal(s.astype(np.int64), src)
        and np.array_equal(d.astype(np.int64), dst)
    )


def _assemble(results):
    full = np.concatenate([results[c]["logits"] for c in range(NC)], axis=0)
    out = np.empty((N, V), np.float32)
    for b in range(B):
        out[b * T:(b + 1) * T] = full[:, b * NCH:b * NCH + T].T
    return out.reshape(B, T, V)


def kernel(**inputs):
    if not _edges_are_block_diag(inputs):
        return _numpy_forward(inputs)
    from concourse import bass2jax

    in_maps = _host_inputs(inputs)
    results = bass2jax.run_bass_via_pjrt(_get_nc(), in_maps, n_cores=NC)
    return _assemble(results)


# --------------------------------------------------------------------------
# benchmarking (repeated execution, device-resident inputs)
# --------------------------------------------------------------------------

def _make_runner(nc):
    """Persistent jitted shard_map callable for nc (multi-core), mirroring
    bass2jax.run_bass_via_pjrt but reusable across calls."""
    import jax
    from jax.sharding import Mesh, PartitionSpec
    from jax.experimental.shard_map import shard_map
    from concourse import bass2jax, mybir as _mybir

    bass2jax.install_neuronx_cc_hook()
    partition_name = nc.partition_id_tensor.name if nc.partition_id_tensor else None
    in_names, out_names, out_avals, zero_outs = [], [], [], []
    for alloc in nc.m.functions[0].allocations:
        if not isinstance(alloc, _mybir.MemoryLocationSet):
            continue
        name = alloc.memorylocations[0].name
        if alloc.kind == "ExternalInput":
            if name != partition_name:
                in_names.append(name)
        elif alloc.kind == "ExternalOutput":
            shape = tuple(alloc.tensor_shape)
            dtype = _mybir.dt.np(alloc.dtype)
            out_names.append(name)
            out_avals.append(jax.core.ShapedArray(shape, dtype))
            zero_outs.append(np.zeros(shape, dtype))
    n_params = len(in_names)
    all_in_names = list(in_names) + list(out_names)
    if partition_name is not None:
        all_in_names.append(partition_name)

    def _body(*args):
        operands = list(args)
        if partition_name is not None:
            operands.append(bass2jax.partition_id_tensor())
        return tuple(
            bass2jax._bass_exec_p.bind(
                *operands,
                out_avals=tuple(out_avals),
                in_names=tuple(all_in_names),
                out_names=tuple(out_names),
                lowering_input_output_aliases=(),
                sim_require_finite=True,
                sim_require_nnan=True,
                nc=nc,
            )
        )

    devices = jax.devices()[:NC]
    mesh = Mesh(np.asarray(devices), ("core",))
    n_outs = len(out_names)
    in_specs = (PartitionSpec("core"),) * (n_params + n_outs)
    out_specs = (PartitionSpec("core"),) * n_outs
    donate = tuple(range(n_params, n_params + n_outs))
    fn = jax.jit(
        shard_map(_body, mesh=mesh, in_specs=in_specs, out_specs=out_specs,
                  check_rep=False),
        donate_argnums=donate, keep_unused=True,
    )
    return fn, in_names, out_names, zero_outs, mesh


def _timed_run(nc, in_maps, iters):
    """Median wall time (s) per execution with device-resident inputs."""
    import jax

    from jax.sharding import NamedSharding, PartitionSpec

    fn, in_names, out_names, zero_outs, mesh = _make_runner(nc)
    shard = NamedSharding(mesh, PartitionSpec("core"))
    concat_in = [
        np.concatenate([np.asarray(m[name]) for m in in_maps], axis=0)
        for name in in_names
    ]
    dev_in = [jax.device_put(a, shard) for a in concat_in]
    jax.block_until_ready(dev_in)

    def zeros():
        zs = [
            jax.device_put(
                np.zeros((NC * z.shape[0], *z.shape[1:]), z.dtype), shard
            )
            for z in zero_outs
        ]
        jax.block_until_ready(zs)
        return zs

    outs = fn(*dev_in, *zeros())  # warm-up/compile
    jax.block_until_ready(outs)
    times = []
    for _ in range(iters):
        zs = zeros()
        t0 = time.perf_counter()
        outs = fn(*dev_in, *zs)
        jax.block_until_ready(outs)
        times.append(time.perf_counter() - t0)
    return float(np.min(times)), outs, out_names


def bench(inputs, iters=16):
    """HW ns per network pass via reps-differential (cancels dispatch cost)."""
    in_maps = _host_inputs(inputs)
    t1, _, _ = _timed_run(_get_nc(1), in_maps, iters)
    t9, _, _ = _timed_run(_get_nc(9), in_maps, iters)
    print(f"  wall/iter reps1: {t1 * 1e6:.0f} us,  reps9: {t9 * 1e6:.0f} us")
    return max(t9 - t1, 0.0) / 8 * 1e9


# revision 26
# speedup vs baseline: 1.4744x; 1.4182x over previous
"""Trainium2 Bass kernel for nn_GAT_42786464203341.

8-way tensor parallel (Megatron-style) over one trn2 chip:
  - GAT edges are block-diagonal fully-connected per sample, so message
    passing is dense per-sample attention with scores leaky(el[i] + er[j]),
    softmaxed over source i (exp needs no max-subtraction, scores O(1)).
  - Activations feature-major (x^T: [D, nodes]); both samples fused on the
    free axis ([128, 532] SBUF tiles, per-266 psum chunks).
  - LayerNorm gains are folded into the weights host-side; the mean term and
    LN bias enter each consumer GEMM as K=1 rank-1 correction matmuls, so LN
    application is a single fused scale-and-cast (x * rsigma -> fp16) per
    k-tile instead of mul+sub+affine.
  - All weights fp16 (full PE rate, FWL weight loads, half the HBM bytes).
    Attention inner math stays f32r.  Matmul accumulation is fp32 PSUM.
  - Attention head-parallel (2 heads/core); W_proj row-sharded -> partial
    [D, nodes] -> AllReduce (fp16).  FFN column/row sharded -> AllReduce.
    Head vocab-sharded; host concatenates the 8 logits slices.
"""

import time
from contextlib import ExitStack

import ml_dtypes
import numpy as np

import concourse.bass as bass
import concourse.tile as tile
from concourse import bacc, mybir
from concourse.masks import make_identity

F32 = mybir.dt.float32
F32R = mybir.dt.float32r
F16 = mybir.dt.float16

B, T, NOBJ = 2, 265, 9
D, H, DH = 1536, 16, 96
V, PV, L, FF = 8192, 512, 3, 6144
N = B * T          # 530
NC = 8             # cores
HPC = H // NC      # heads per core
FFL = FF // NC     # 768
VL = V // NC       # 1024
NCH = T + 1        # 266 (col 265 of each chunk is zero padding)
NP = B * NCH       # 532
KD = D // 128      # 12
KF = FFL // 128    # 6
MTS = [[(0, 128), (128, 128), (256, 10)],
       [(266, 128), (394, 128), (522, 10)]]    # node tiles (fused offsets)
MT_REAL = [128, 128, 9]                        # non-pad rows per node tile
EPS = 1e-5

_CACHE = {}


# --------------------------------------------------------------------------
# host-side input prep
# --------------------------------------------------------------------------

def _block_diag_edges_np():
    base = np.arange(T)
    src = np.concatenate([g * T + np.repeat(base, T) for g in range(B)])
    dst = np.concatenate([g * T + np.tile(base, T) for g in range(B)])
    return src.astype(np.int64), dst.astype(np.int64)


def _host_inputs(inp, ffn_bf16=True):
    f32, f16 = np.float32, np.float16
    objs_e = np.asarray(inp["obj_emb_w"])[np.asarray(inp["objs"])]
    pe = np.asarray(inp["poss_emb_w"])[np.asarray(inp["poss"])]
    nfeat = np.concatenate([objs_e, pe[:, :NOBJ], pe[:, NOBJ:]], axis=-1)
    z = np.asarray(inp["tok_emb"])[np.asarray(inp["z_indices"])]
    x0 = np.concatenate([nfeat, z], axis=1) + np.asarray(inp["pos_emb"])[:, :T]
    x0 = x0.reshape(N, D).astype(f32)

    x0t = np.zeros((D, NP), f16)
    for b in range(B):
        x0t[:, b * NCH:b * NCH + T] = x0[b * T:(b + 1) * T].T.astype(f16)

    W_attn = np.asarray(inp["W_attn"], f32)
    a_l = np.asarray(inp["a_l"], f32)
    a_r = np.asarray(inp["a_r"], f32)
    W_proj = np.asarray(inp["W_proj"], f32)
    W_fc = np.asarray(inp["W_fc"], f32)
    W_out = np.asarray(inp["W_out"], f32)
    head_w = np.asarray(inp["head_w"], f32)
    g1 = np.asarray(inp["ln1_g"], f32)      # [L, D]
    b1 = np.asarray(inp["ln1_b"], f32)
    g2 = np.asarray(inp["ln2_g"], f32)
    b2 = np.asarray(inp["ln2_b"], f32)
    gf = np.asarray(inp["lnf_g"], f32)      # [D]
    bf = np.asarray(inp["lnf_b"], f32)

    def cols(vec, k_tiles):  # [3, D'] -> [3, 128, k_tiles]
        v = np.asarray(vec, f32)
        return np.transpose(v.reshape(3, k_tiles, 128), (0, 2, 1)).copy()

    bout8 = cols(np.asarray(inp["b_out"], f32) / NC, KD)
    bproj8 = cols(np.asarray(inp["b_proj"], f32) / NC, KD)

    maps = []
    for c in range(NC):
        h0 = c * HPC
        wattn = np.zeros((L, D, 256), f32)
        for j in range(HPC):
            hg = h0 + j
            blk = W_attn[:, :, hg * DH:(hg + 1) * DH]         # [3, D, DH]
            wattn[:, :, j * DH:(j + 1) * DH] = blk
            # el/er are linear in h: fold (W_attn-block @ a) into one column
            wattn[:, :, 192 + j] = np.matmul(blk, a_l[:, hg, :, None])[..., 0]
            wattn[:, :, 194 + j] = np.matmul(blk, a_r[:, hg, :, None])[..., 0]
        # LN1 fold: gain into weights; Gsum/Beta correction rows
        wattn_g = wattn * g1[:, :, None]                      # [L, D, 256]
        wa_gsum = np.einsum("ld,ldc->lc", g1, wattn)          # [L, 256]
        wa_beta = np.einsum("ld,ldc->lc", b1, wattn)          # [L, 256]

        wproj = np.stack(
            [W_proj[:, (h0 + j) * DH:(h0 + j + 1) * DH, :] for j in range(HPC)],
            axis=1,
        )
        wfc_sl = W_fc[:, :, c * FFL:(c + 1) * FFL]            # [L, D, FFL]
        wfc_g = wfc_sl * g2[:, :, None]
        wfc_gsum = np.einsum("ld,ldm->lm", g2, wfc_sl)        # [L, FFL]
        # LN2 beta folds into the fc bias column
        bfc_eff = np.asarray(inp["b_fc"], f32)[:, c * FFL:(c + 1) * FFL] \
            + np.einsum("ld,ldm->lm", b2, wfc_sl)             # [L, FFL]
        bfc_cols = np.transpose(bfc_eff.reshape(L, KF, 128), (0, 2, 1)).copy()

        whead_sl = head_w[:, c * VL:(c + 1) * VL]             # [D, VL]
        whead_g = whead_sl * gf[:, None]
        whead_gsum = (gf @ whead_sl)[None, :]                 # [1, VL]
        headb = (bf @ whead_sl).reshape(VL // 128, 128).T.copy()  # [128, 8]

        maps.append({
            "x0t": x0t,
            "wattn": wattn_g.astype(f16),
            "wa_gsum": wa_gsum[:, None, :].astype(f16),       # [L, 1, 256]
            "wa_beta": wa_beta[:, None, :].astype(f16),
            "wproj": np.ascontiguousarray(wproj).astype(f16),
            "wfc": np.ascontiguousarray(wfc_g).astype(f16),
            "wfc_gsum": wfc_gsum[:, None, :].astype(f16),     # [L, 1, FFL]
            "wout": np.ascontiguousarray(W_out[:, c * FFL:(c + 1) * FFL, :]).astype(f16),
            "whead": np.ascontiguousarray(whead_g).astype(f16),
            "whead_gsum": whead_gsum.astype(f16),             # [1, VL]
            "headb": headb,                                   # [128, 8] f32
            "ones_col": np.ones((128, 1), f32),
            "ones_row": np.ones((1, 128), f32),
            "bfc": np.ascontiguousarray(bfc_cols[..., None]), # [L, 128, KF, 1]->?
            "bout8": bout8, "bproj8": bproj8,
        })
    # fix bfc shape: want [L, KF, 128, 1]
    for m in maps:
        b = m["bfc"][..., 0]                                  # [L, 128, KF]
        m["bfc"] = np.ascontiguousarray(np.transpose(b, (0, 2, 1))[..., None])
    return maps


# --------------------------------------------------------------------------
# device program
# --------------------------------------------------------------------------

def _build_nc(reps=1, use_cc=True, ffn_bf16=True):
    nc = bacc.Bacc("TRN2", target_bir_lowering=False, debug=False, num_devices=NC)

    d_x0t = nc.declare_dram_parameter("x0t", [D, NP], F16, isOutput=False)
    d_wattn = nc.declare_dram_parameter("wattn", [L, D, 256], F16, isOutput=False)
    d_wagsum = nc.declare_dram_parameter("wa_gsum", [L, 1, 256], F16, isOutput=False)
    d_wabeta = nc.declare_dram_parameter("wa_beta", [L, 1, 256], F16, isOutput=False)
    d_wproj = nc.declare_dram_parameter("wproj", [L, HPC, DH, D], F16, isOutput=False)
    d_wfc = nc.declare_dram_parameter("wfc", [L, D, FFL], F16, isOutput=False)
    d_wfcgsum = nc.declare_dram_parameter("wfc_gsum", [L, 1, FFL], F16, isOutput=False)
    d_wout = nc.declare_dram_parameter("wout", [L, FFL, D], F16, isOutput=False)
    d_whead = nc.declare_dram_parameter("whead", [D, VL], F16, isOutput=False)
    d_whgsum = nc.declare_dram_parameter("whead_gsum", [1, VL], F16, isOutput=False)
    d_headb = nc.declare_dram_parameter("headb", [128, VL // 128], F32, isOutput=False)
    d_ones_col = nc.declare_dram_parameter("ones_col", [128, 1], F32R, isOutput=False)
    d_ones_row = nc.declare_dram_parameter("ones_row", [1, 128], F32R, isOutput=False)
    d_bfc = nc.declare_dram_parameter("bfc", [L, KF, 128, 1], F32, isOutput=False)
    d_bout8 = nc.declare_dram_parameter("bout8", [L, 128, KD], F32, isOutput=False)
    d_bproj8 = nc.declare_dram_parameter("bproj8", [L, 128, KD], F32, isOutput=False)
    d_logits = nc.declare_dram_parameter("logits", [VL, NP], F32, isOutput=True)

    ar_in, ar_out = {}, {}
    for l in range(L):
        for s in range(2):
            ar_in[l, s] = nc.dram_tensor(f"arin_{l}_{s}", [D, NP], F16)
            ar_out[l, s] = nc.dram_tensor(
                f"arout_{l}_{s}", [D, NP], F16, addr_space="Shared"
            )

    AF = mybir.ActivationFunctionType
    ALU = mybir.AluOpType

    with tile.TileContext(nc) as tc, ExitStack() as ctx:
        res = ctx.enter_context(tc.tile_pool(name="res", bufs=1))
        cst = ctx.enter_context(tc.tile_pool(name="cst", bufs=2))
        a1 = ctx.enter_context(tc.tile_pool(name="a1", bufs=2))
        a2 = ctx.enter_context(tc.tile_pool(name="a2", bufs=2))
        a3 = ctx.enter_context(tc.tile_pool(name="a3", bufs=3))
        wgt = ctx.enter_context(tc.tile_pool(name="wgt", bufs=1))
        psR = ctx.enter_context(tc.tile_pool(name="psR", bufs=2, space="PSUM"))
        psB = ctx.enter_context(tc.tile_pool(name="psB", bufs=2, space="PSUM"))
        psM = ctx.enter_context(tc.tile_pool(name="psM", bufs=3, space="PSUM"))

        sqp = ctx.enter_context(tc.tile_pool(name="sqp", bufs=1))

        ones_col = res.tile([128, 1], F32R, tag="ones_col")
        nc.sync.dma_start(out=ones_col[:], in_=d_ones_col[:])
        ones_col16 = res.tile([128, 1], F16, tag="ones_col16")
        nc.vector.memset(ones_col16[:], 1.0)
        ones_row = res.tile([1, 128], F32R, tag="ones_row")
        nc.sync.dma_start(out=ones_row[:], in_=d_ones_row[:])
        ones_f16 = res.tile([1, NP], F16, tag="ones_f16")
        nc.vector.memset(ones_f16[:], 1.0)
        ident = res.tile([128, 128], F32, tag="ident")
        make_identity(nc, ident[:])
        eps_col = res.tile([1, 1], F32, tag="eps")
        nc.vector.memset(eps_col[:], EPS)

        def layer_norm(xt, corr_tag):
            """Fused-batch LN stats: returns (xs 12x[128,NP] f16 scaled tiles,
            mrs [1,NP] f16 row of -mean*rsigma)."""
            mrs = a1.tile([1, NP], F16, tag="mrs")
            sq_tiles = []
            for k in range(KD):
                sq = sqp.tile([128, NP], F16, tag=f"sq{k}")
                with nc.allow_low_precision("fp16 sumsq"):
                    nc.scalar.activation(sq[:], xt[k][:], AF.Square)
                sq_tiles.append(sq)
            bcs = []
            for s in range(2):
                c0 = s * NCH
                p_sums = psR.tile([1, NCH], F32, tag="row")
                for k in range(KD):
                    nc.tensor.matmul(
                        p_sums[:], ones_col16[:], xt[k][:, c0:c0 + NCH],
                        start=(k == 0), stop=(k == KD - 1),
                    )
                p_sqs = psR.tile([1, NCH], F32, tag="row")
                for k in range(KD):
                    nc.tensor.matmul(
                        p_sqs[:], ones_col16[:], sq_tiles[k][:, c0:c0 + NCH],
                        start=(k == 0), stop=(k == KD - 1),
                    )
                m_row = a1.tile([1, NCH], F32, tag="m_row")
                nc.vector.tensor_scalar(m_row[:], p_sums[:], 1.0 / D, None, ALU.mult)
                ms = a1.tile([1, NCH], F32, tag="ms_row")
                nc.vector.tensor_mul(ms[:], m_row[:], m_row[:])
                var = a1.tile([1, NCH], F32, tag="var_row")
                nc.vector.scalar_tensor_tensor(
                    var[:], p_sqs[:], 1.0 / D, ms[:], ALU.mult, ALU.subtract
                )
                std = a1.tile([1, NCH], F32, tag="std_row")
                nc.scalar.activation(std[:], var[:], AF.Sqrt, bias=eps_col[:])
                rs_row = a1.tile([1, NCH], F32R, tag="rs_row")
                with nc.allow_low_precision("f32r rounding"):
                    nc.vector.reciprocal(rs_row[:], std[:])
                with nc.allow_low_precision("fp16 correction row"):
                    nc.vector.scalar_tensor_tensor(
                        mrs[:, c0:c0 + NCH], m_row[:], -1.0,
                        rs_row[:].bitcast(F32), ALU.mult, ALU.mult,
                    )
                p_bc = psB.tile([128, NCH], F32, tag="bc")
                nc.tensor.matmul(p_bc[:], ones_row[:], rs_row[:], start=True, stop=True)
                bcs.append(p_bc)
            xs_tiles = []
            for k in range(KD):
                xs = sqp.tile([128, NP], F16, name=f"xs{k}", tag=f"xs{k}")
                for s in range(2):
                    c0 = s * NCH
                    with nc.allow_low_precision("fp16 matmul input"):
                        nc.vector.tensor_mul(
                            xs[:, c0:c0 + NCH], xt[k][:, c0:c0 + NCH],
                            bcs[s][:],
                        )
                xs_tiles.append(xs)
            return xs_tiles, mrs

        def partial_out(psum, b8_sb, mi, c0, part, on_act):
            with nc.allow_low_precision("fp16 allreduce payload"):
                if on_act:
                    nc.scalar.activation(
                        part[:, c0:c0 + NCH], psum[:], AF.Identity,
                        bias=b8_sb[:, mi:mi + 1],
                    )
                else:
                    nc.vector.tensor_scalar(
                        part[:, c0:c0 + NCH], psum[:], b8_sb[:, mi:mi + 1],
                        None, ALU.add,
                    )

        def all_reduce(l, s):
            if use_cc:
                nc.gpsimd.collective_compute(
                    "AllReduce", ALU.add,
                    replica_groups=[list(range(NC))],
                    ins=[ar_in[l, s][:].opt()],
                    outs=[ar_out[l, s][:].opt()],
                )
            else:
                nc.gpsimd.dma_start(out=ar_out[l, s][:], in_=ar_in[l, s][:])

        def refresh_xt(xt, l, s):
            for k in range(KD):
                tmp = a3.tile([128, NP], F16, tag="artmp")
                nc.gpsimd.dma_start(
                    out=tmp[:], in_=ar_out[l, s][k * 128:(k + 1) * 128, :]
                )
                with nc.allow_low_precision("fp16 residual"):
                    nc.vector.tensor_add(xt[k][:], xt[k][:], tmp[:])

        for _rep in range(reps):
            xt = []
            for k in range(KD):
                t = res.tile([128, NP], F16, name=f"xt{k}", tag=f"xt{k}")
                nc.sync.dma_start(
                    out=t[:], in_=d_x0t[k * 128:(k + 1) * 128, :]
                )
                xt.append(t)

            for l in range(L):
                bfc_t = []
                for mi in range(KF):
                    bt = cst.tile([128, 1], F32, name=f"bfc{mi}", tag=f"bfc{mi}")
                    nc.sync.dma_start(out=bt[:], in_=d_bfc[l, mi])
                    bfc_t.append(bt)
                bout8_sb = cst.tile([128, KD], F32, tag="bout8")
                nc.sync.dma_start(out=bout8_sb[:], in_=d_bout8[l])
                bproj8_sb = cst.tile([128, KD], F32, tag="bproj8")
                nc.sync.dma_start(out=bproj8_sb[:], in_=d_bproj8[l])

                wa = []
                for k in range(KD):
                    t = wgt.tile([128, 256], F16, tag=f"wa{k}")
                    nc.sync.dma_start(
                        out=t[:], in_=d_wattn[l, k * 128:(k + 1) * 128, :]
                    )
                    wa.append(t)
                wagsum = cst.tile([1, 256], F16, tag="wagsum")
                nc.sync.dma_start(out=wagsum[:], in_=d_wagsum[l])
                wabeta = cst.tile([1, 256], F16, tag="wabeta")
                nc.sync.dma_start(out=wabeta[:], in_=d_wabeta[l])
                wp = []
                for j in range(HPC):
                    t = wgt.tile([DH, D], F16, tag=f"wp{j}")
                    nc.sync.dma_start(out=t[:], in_=d_wproj[l, j])
                    wp.append(t)

                # ---------- attention sublayer ----------
                xs, mrs = layer_norm(xt, f"l{l}a")

                whsb = [[None] * 3 for _ in range(2)]
                for s in range(2):
                    for mi, (ms, msz) in enumerate(MTS[s]):
                        p = psM.tile([128, 256], F32, tag="mm")
                        for k in range(KD):
                            nc.tensor.matmul(
                                p[:msz, :], xs[k][:, ms:ms + msz], wa[k][:],
                                start=(k == 0), stop=False,
                            )
                        nc.tensor.matmul(
                            p[:msz, :], mrs[:, ms:ms + msz], wagsum[:],
                            start=False, stop=False,
                        )
                        nc.tensor.matmul(
                            p[:msz, :], ones_f16[:, ms:ms + msz], wabeta[:],
                            start=False, stop=True,
                        )
                        w = a1.tile([128, 198], F32R, tag=f"whsb{mi}_{s}")
                        nc.scalar.copy(w[:msz, 0:196], p[:msz, 0:196])
                        nc.vector.tensor_scalar(
                            w[:msz, 196:198], p[:msz, 192:194], 0.2, None, ALU.mult
                        )
                        whsb[s][mi] = w

                aggt = [
                    a1.tile([DH, NP], F16, name=f"aggt{j}", tag=f"aggt{j}")
                    for j in range(HPC)
                ]
                for s in range(2):
                    erow = [
                        a1.tile([1, NCH], F32R, name=f"er{j}_{s}", tag=f"er{j}_{s}")
                        for j in range(HPC)
                    ]
                    for mi, (ms, msz) in enumerate(MTS[s]):
                        for j in range(HPC):
                            pt = psR.tile([1, 128], F32, tag="row")
                            nc.tensor.transpose(
                                pt[:, :msz],
                                whsb[s][mi][:msz, 194 + j:195 + j].bitcast(F32),
                                ident[:msz, :msz],
                            )
                            lo = ms - s * NCH
                            nc.scalar.copy(erow[j][:, lo:lo + msz], pt[:, :msz])

                    for j in range(HPC):
                        p_er = psB.tile([128, NCH], F32, tag="bc")
                        nc.tensor.matmul(
                            p_er[:], ones_row[:], erow[j][:], start=True, stop=True
                        )
                        e_tiles = []
                        for mi in range(3):
                            rsz = MT_REAL[mi]
                            e1 = a2.tile([128, NCH], F32R, tag=f"e{mi}")
                            nc.scalar.activation(
                                e1[:rsz, :], p_er[:rsz, :], AF.Exp,
                                bias=whsb[s][mi][:rsz, 192 + j:193 + j].bitcast(F32),
                            )
                            e2 = a2.tile([128, NCH], F32, tag="e2")
                            nc.scalar.activation(
                                e2[:rsz, :], p_er[:rsz, :], AF.Exp, scale=0.2,
                                bias=whsb[s][mi][:rsz, 196 + j:197 + j].bitcast(F32),
                            )
                            nc.vector.tensor_max(
                                e1[:rsz, :], e1[:rsz, :].bitcast(F32), e2[:rsz, :]
                            )
                            e_tiles.append(e1)
                        p_s = psR.tile([1, NCH], F32, tag="row")
                        for mi in range(3):
                            rsz = MT_REAL[mi]
                            nc.tensor.matmul(
                                p_s[:], ones_col[:rsz, :], e_tiles[mi][:rsz, :],
                                start=(mi == 0), stop=(mi == 2),
                            )
                        r_row = a1.tile([1, NCH], F32R, tag="r_row")
                        with nc.allow_low_precision("f32r rounding"):
                            nc.vector.reciprocal(r_row[:], p_s[:])
                        p_rb2 = psB.tile([DH, NCH], F32, tag="bc")
                        nc.tensor.matmul(
                            p_rb2[:], ones_row[:, :DH], r_row[:], start=True, stop=True
                        )
                        rb_sb = a1.tile([DH, NCH], F32, tag="rb_sb")
                        nc.scalar.copy(rb_sb[:], p_rb2[:])
                        p_agg = psM.tile([DH, NCH], F32, tag="mm")
                        for mi in range(3):
                            rsz = MT_REAL[mi]
                            nc.tensor.matmul(
                                p_agg[:],
                                whsb[s][mi][:rsz, j * DH:(j + 1) * DH],
                                e_tiles[mi][:rsz, :],
                                start=(mi == 0), stop=(mi == 2),
                            )
                        c0 = s * NCH
                        with nc.allow_low_precision("fp16 agg"):
                            nc.vector.tensor_mul(
                                aggt[j][:, c0:c0 + NCH], p_agg[:], rb_sb[:]
                            )

                for mi in range(KD):
                    part = a3.tile([128, NP], F16, tag="part")
                    for s in range(2):
                        c0 = s * NCH
                        p = psM.tile([128, NCH], F32, tag="mm")
                        for j in range(HPC):
                            nc.tensor.matmul(
                                p[:], wp[j][:, mi * 128:(mi + 1) * 128],
                                aggt[j][:, c0:c0 + NCH],
                                start=(j == 0), stop=(j == HPC - 1),
                            )
                        partial_out(p, bproj8_sb, mi, c0, part, on_act=False)
                    nc.sync.dma_start(
                        out=ar_in[l, 0][mi * 128:(mi + 1) * 128, :], in_=part[:]
                    )
                all_reduce(l, 0)
                refresh_xt(xt, l, 0)

                # ---------- FFN sublayer ----------
                wfc_sb = []
                for k in range(KD):
                    t = wgt.tile([128, VL], F16, tag=f"wbig{k}")
                    nc.sync.dma_start(
                        out=t[:, 0:FFL], in_=d_wfc[l, k * 128:(k + 1) * 128, :]
                    )
                    wfc_sb.append(t)
                wfcgsum = cst.tile([1, FFL], F16, tag="wfcgsum")
                nc.sync.dma_start(out=wfcgsum[:], in_=d_wfcgsum[l])
                wout_sb = []
                for k in range(KF):
                    t = wgt.tile([128, D], F16, tag=f"wo{k}")
                    nc.sync.dma_start(
                        out=t[:], in_=d_wout[l, k * 128:(k + 1) * 128, :]
                    )
                    wout_sb.append(t)

                xs2, mrs2 = layer_norm(xt, f"l{l}f")
                g_tiles = []
                for mi in range(KF):
                    g = a2.tile([128, NP], F16, tag=f"g{mi}")
                    for s in range(2):
                        c0 = s * NCH
                        p = psM.tile([128, NCH], F32, tag="mm")
                        for k in range(KD):
                            nc.tensor.matmul(
                                p[:], wfc_sb[k][:, mi * 128:(mi + 1) * 128],
                                xs2[k][:, c0:c0 + NCH],
                                start=(k == 0), stop=False,
                            )
                        nc.tensor.matmul(
                            p[:], wfcgsum[:, mi * 128:(mi + 1) * 128],
                            mrs2[:, c0:c0 + NCH], start=False, stop=True,
                        )
                        nc.scalar.activation(
                            g[:, c0:c0 + NCH], p[:], AF.Gelu, bias=bfc_t[mi][:]
                        )
                    g_tiles.append(g)
                for mi in range(KD):
                    part = a3.tile([128, NP], F16, tag="part")
                    for s in range(2):
                        c0 = s * NCH
                        p = psM.tile([128, NCH], F32, tag="mm")
                        for k in range(KF):
                            nc.tensor.matmul(
                                p[:], wout_sb[k][:, mi * 128:(mi + 1) * 128],
                                g_tiles[k][:, c0:c0 + NCH],
                                start=(k == 0), stop=(k == KF - 1),
                            )
                        partial_out(p, bout8_sb, mi, c0, part, on_act=False)
                    nc.sync.dma_start(
                        out=ar_in[l, 1][mi * 128:(mi + 1) * 128, :], in_=part[:]
                    )
                all_reduce(l, 1)
                refresh_xt(xt, l, 1)

            # ---------- final LN + vocab-sharded head ----------
            wh_sb = []
            for k in range(KD):
                t = wgt.tile([128, VL], F16, name=f"whd{k}", tag=f"wbig{k}")
                nc.sync.dma_start(out=t[:], in_=d_whead[k * 128:(k + 1) * 128, :])
                wh_sb.append(t)
            whgsum = cst.tile([1, VL], F16, tag="whgsum")
            nc.sync.dma_start(out=whgsum[:], in_=d_whgsum[:])
            headb_sb = cst.tile([128, VL // 128], F32, tag="headb")
            nc.sync.dma_start(out=headb_sb[:], in_=d_headb[:])

            xsf, mrsf = layer_norm(xt, "lnf")
            for mi in range(VL // 128):
                lg = a3.tile([128, NP], F32, tag="lg")
                for s in range(2):
                    c0 = s * NCH
                    p = psM.tile([128, NCH], F32, tag="mm")
                    for k in range(KD):
                        nc.tensor.matmul(
                            p[:], wh_sb[k][:, mi * 128:(mi + 1) * 128],
                            xsf[k][:, c0:c0 + NCH],
                            start=(k == 0), stop=False,
                        )
                    nc.tensor.matmul(
                        p[:], whgsum[:, mi * 128:(mi + 1) * 128],
                        mrsf[:, c0:c0 + NCH], start=False, stop=True,
                    )
                    nc.scalar.activation(
                        lg[:, c0:c0 + NCH], p[:], AF.Identity,
                        bias=headb_sb[:, mi:mi + 1],
                    )
                nc.sync.dma_start(
                    out=d_logits[mi * 128:(mi + 1) * 128, :], in_=lg[:]
                )

    nc.compile()
    return nc


def _get_nc(reps=1, use_cc=True, ffn_bf16=True):
    key = f"nc{reps}_{use_cc}_{ffn_bf16}"
    if key not in _CACHE:
        _CACHE[key] = _build_nc(reps, use_cc, ffn_bf16)
    return _CACHE[key]


# --------------------------------------------------------------------------
# numpy fallback (exact reference semantics for arbitrary edges)
# --------------------------------------------------------------------------

def _numpy_forward(inp):
    from scipy.special import erf

    def ln(x, g, b):
        m = x.mean(-1, keepdims=True)
        v = ((x - m) ** 2).mean(-1, keepdims=True)
        return (x - m) / np.sqrt(v + EPS) * g + b

    f32 = np.float32
    objs_e = np.asarray(inp["obj_emb_w"])[np.asarray(inp["objs"])]
    pe = np.asarray(inp["poss_emb_w"])[np.asarray(inp["poss"])]
    nfeat = np.concatenate([objs_e, pe[:, :NOBJ], pe[:, NOBJ:]], axis=-1)
    z = np.asarray(inp["tok_emb"])[np.asarray(inp["z_indices"])]
    x = np.concatenate([nfeat, z], axis=1) + np.asarray(inp["pos_emb"])[:, :T]
    x = x.reshape(N, D).astype(f32)
    src = np.asarray(inp["src"]).astype(np.int64)
    dst = np.asarray(inp["dst"]).astype(np.int64)
    for l in range(L):
        h = ln(x, inp["ln1_g"][l], inp["ln1_b"][l])
        Wh = (h @ np.asarray(inp["W_attn"][l])).reshape(N, H, DH)
        el = np.einsum("nhd,hd->nh", Wh, np.asarray(inp["a_l"][l]))
        er = np.einsum("nhd,hd->nh", Wh, np.asarray(inp["a_r"][l]))
        e = el[src] + er[dst]
        e = np.where(e >= 0, e, 0.2 * e)
        m = np.full((N, H), -np.inf, f32)
        np.maximum.at(m, dst, e)
        m[~np.isfinite(m)] = 0.0
        ex = np.exp(e - m[dst])
        s = np.zeros((N, H), f32)
        np.add.at(s, dst, ex)
        alpha = ex / s[dst]
        agg = np.zeros((N, H, DH), f32)
        np.add.at(agg, dst, alpha[:, :, None] * Wh[src])
        x = x + agg.reshape(N, D) @ np.asarray(inp["W_proj"][l]) \
            + np.asarray(inp["b_proj"][l])
        h2 = ln(x, inp["ln2_g"][l], inp["ln2_b"][l])
        ff = h2 @ np.asarray(inp["W_fc"][l]) + np.asarray(inp["b_fc"][l])
        ff = ff * 0.5 * (1.0 + erf(ff / np.sqrt(2.0)))
        x = x + ff @ np.asarray(inp["W_out"][l]) + np.asarray(inp["b_out"][l])
    x = ln(x, inp["lnf_g"], inp["lnf_b"])
    return (x @ np.asarray(inp["head_w"])).reshape(B, T, V).astype(f32)


# --------------------------------------------------------------------------
# public entry
# --------------------------------------------------------------------------

def _edges_are_block_diag(inp):
    src, dst = _block_diag_edges_np()
    s = np.asarray(inp["src"])
    d = np.asarray(inp["dst"])
    return (
        s.shape == src.shape
        and np.array_equal(s.astype(np.int64), src)
        and np.array_equal(d.astype(np.int64), dst)
    )


def _assemble(results):
    full = np.concatenate([results[c]["logits"] for c in range(NC)], axis=0)
    out = np.empty((N, V), np.float32)
    for b in range(B):
        out[b * T:(b + 1) * T] = full[:, b * NCH:b * NCH + T].T
    return out.reshape(B, T, V)


def kernel(**inputs):
    if not _edges_are_block_diag(inputs):
        return _numpy_forward(inputs)
    from concourse import bass2jax

    in_maps = _host_inputs(inputs)
    results = bass2jax.run_bass_via_pjrt(_get_nc(), in_maps, n_cores=NC)
    return _assemble(results)


# --------------------------------------------------------------------------
# benchmarking (repeated execution, device-resident inputs)
# --------------------------------------------------------------------------

def _make_runner(nc):
    """Persistent jitted shard_map callable for nc (multi-core), mirroring
    bass2jax.run_bass_via_pjrt but reusable across calls."""
    import jax
    from jax.sharding import Mesh, PartitionSpec
    from jax.experimental.shard_map import shard_map
    from concourse import bass2jax, mybir as _mybir

    bass2jax.install_neuronx_cc_hook()
    partition_name = nc.partition_id_tensor.name if nc.partition_id_tensor else None
    in_names, out_names, out_avals, zero_outs = [], [], [], []
    for alloc in nc.m.functions[0].allocations:
        if not isinstance(alloc, _mybir.MemoryLocationSet):
            continue
        name = alloc.memorylocations[0].name
        if alloc.kind == "ExternalInput":
            if name != partition_name:
                in_names.append(name)
        elif alloc.kind == "ExternalOutput":
            shape = tuple(alloc.tensor_shape)
            dtype = _mybir.dt.np(alloc.dtype)
            out_names.append(name)
            out_avals.append(jax.core.ShapedArray(shape, dtype))
            zero_outs.append(np.zeros(shape, dtype))
    n_params = len(in_names)
    all_in_names = list(in_names) + list(out_names)
    if partition_name is not None:
        all_in_names.append(partition_name)

    def _body(*args):
        operands = list(args)
        if partition_name is not None:
            operands.append(bass2jax.partition_id_tensor())
        return tuple(
            bass2jax._bass_exec_p.bind(
                *operands,
                out_avals=tuple(out_avals),
                in_names=tuple(all_in_names),
                out_names=tuple(out_names),
                lowering_input_output_aliases=(),
                sim_require_finite=True,
                sim_require_nnan=True,
                nc=nc,
            )
        )

    devices = jax.devices()[:NC]
    mesh = Mesh(np.asarray(devices), ("core",))
    n_outs = len(out_names)
    in_specs = (PartitionSpec("core"),) * (n_params + n_outs)
    out_specs = (PartitionSpec("core"),) * n_outs
    donate = tuple(range(n_params, n_params + n_outs))
    fn = jax.jit(
        shard_map(_body, mesh=mesh, in_specs=in_specs, out_specs=out_specs,
                  check_rep=False),
        donate_argnums=donate, keep_unused=True,
    )
    return fn, in_names, out_names, zero_outs, mesh


def _timed_run(nc, in_maps, iters):
    """Median wall time (s) per execution with device-resident inputs."""
    import jax

    from jax.sharding import NamedSharding, PartitionSpec

    fn, in_names, out_names, zero_outs, mesh = _make_runner(nc)
    shard = NamedSharding(mesh, PartitionSpec("core"))
    concat_in = [
        np.concatenate([np.asarray(m[name]) for m in in_maps], axis=0)
        for name in in_names
    ]
    dev_in = [jax.device_put(a, shard) for a in concat_in]
    jax.block_until_ready(dev_in)

    def zeros():
        zs = [
            jax.device_put(
                np.zeros((NC * z.shape[0], *z.shape[1:]), z.dtype), shard
            )
            for z in zero_outs
        ]
        jax.block_until_ready(zs)
        return zs

    outs = fn(*dev_in, *zeros())  # warm-up/compile
    jax.block_until_ready(outs)
    times = []
    for _ in range(iters):
        zs = zeros()
        t0 = time.perf_counter()
        outs = fn(*dev_in, *zs)
        jax.block_until_ready(outs)
        times.append(time.perf_counter() - t0)
    return float(np.min(times)), outs, out_names


def bench(inputs, iters=16):
    """HW ns per network pass via reps-differential (cancels dispatch cost)."""
    in_maps = _host_inputs(inputs)
    t1, _, _ = _timed_run(_get_nc(1), in_maps, iters)
    t9, _, _ = _timed_run(_get_nc(9), in_maps, iters)
    print(f"  wall/iter reps1: {t1 * 1e6:.0f} us,  reps9: {t9 * 1e6:.0f} us")
    return max(t9 - t1, 0.0) / 8 * 1e9
